# revision 34
# baseline (speedup 1.0000x reference)
# Trainium2 Bass kernel for nn_FCM_series_1 (gnn_message_passing).
#
# Math (derived from the reference):
#   aggregate(X, WW)[l,b,j] = tanh(-sum_i X[l,b,i] * WW[i,j])
#   T_A  = aggregate(A, WW)                     (12 lags x B rows)
#   U[t] = aggregate(train_init[:,:,t,1], WW)   (13 unique rows per batch;
#          A_N_OLD[la] = U[la], A_0_NEW[la] = U[la+1])
#   out[b,la,j] = P[la,j]*T_A[la,b,j] + Q[la,j]*U[la+1,b,j] + R[la,j]*U[la,b,j]
# with host-computable coefficients
#   P[la,j] = 2 * lambd[la, j%200] / belta[la] * 3**fract[la]
#   Q[la,j] = 3 * lambd[la, j%200] * l[la, j%200] / belta[la]
#   R[la,j] = Q[la,j] * Gamma(a+1)/(6*Gamma(a-2))
#   belta[la] = sum_{k=0..3} Gamma(a+1)/(Gamma(k+1)*Gamma(a-k+1))
#
# Sharding over 8 cores: batch split x2 (16 each), output node dim j split x4
# (300 each). Per core one matmul chain: lhsT=W-chunk tiles, rhs=X^T tiles,
# PSUM-accumulated over 10 k-tiles of 120, in float32r (single-pass fp32 PE
# mode, 4x faster than fp32 LOW_HIGH). W is negated on the host so psum
# already holds -X@W; tanh on ScalarE; coefficient combine on VectorE with
# 0-stride broadcast APs; per-core [300,192] result re-assembled on the host.
#
# HBM layouts are host-repacked to partition-major so every DMA descriptor is
# one large contiguous run per partition; input DMAs are split between the two
# HWDGE queues (sync for W, scalar for X) to double aggregate DMA throughput.

import math

import numpy as np

LAG = 13
B = 32
N = 1200
H = 1.0 / 3.0

# FOUR_CORES: run on one NeuronCore per SEngine pair (devices 0,2,4,6).
# Paired cores share an HBM stack and the 2:1-muxed SDMA engines, so 8
# concurrent cores each get only ~215 GB/s; spacing 4 cores out gives each
# ~full bandwidth on 4.8 MB/core instead of 3.36 MB/core at half rate.
FOUR_CORES = True
PB = 2          # batch shards
PJ = 2 if FOUR_CORES else 4   # j shards
BL = B // PB    # 16 batches per core
JL = N // PJ    # output nodes per core (600 / 300)
NL = LAG - 1    # 12
CA = NL * BL    # 192 cols: T_A block, col = la*BL + b
CU = LAG * BL   # 208 cols: U block,  col = CA + t*BL + b
C = CA + CU     # 400 matmul moving cols
KT = 120        # contraction tile
NK = N // KT    # 10
JS = 100        # j subtile (psum partition dim)
NJ = JL // JS   # j subtiles per core (6 / 3)
NCH = 2         # input DMA chunks per tensor (5 k-tiles each)
USE_BF16 = False  # matmul operand dtype: bf16 halves DMA bytes, ~2.5e-3 rel err
DEVICE_IDS = [0, 2, 4, 6] if FOUR_CORES else list(range(8))

_cached = None


def _gamma(x):
    return math.gamma(x)


def _build_nc():
    import concourse.bacc as bacc
    import concourse.mybir as mybir
    from concourse.tile import TileContext

    f32 = mybir.dt.float32
    f32r = mybir.dt.bfloat16 if USE_BF16 else mybir.dt.float32r
    nc = bacc.Bacc(None, target_bir_lowering=False)

    # partition-major repacked inputs (see kernel() for layouts)
    xt = nc.dram_tensor("xt", [KT, NK * C], f32r, kind="ExternalInput")
    wc = nc.dram_tensor("wc", [KT, NK * JL], f32r, kind="ExternalInput")
    coef = nc.dram_tensor("coef", [JS, 3 * NJ * NL], f32, kind="ExternalInput")
    out = nc.dram_tensor("out", [JL, CA], f32, kind="ExternalOutput")

    with TileContext(nc) as tc:
        with (
            tc.tile_pool(name="sb", bufs=1) as pool,
            tc.tile_pool(name="ps", bufs=1, space="PSUM") as pspool,
        ):
            x_tiles = [None] * NK   # per-k [KT, C] views
            w_tiles = [None] * NK   # 8-core mode: per-k [KT, JL] views
            wjt = [None] * NJ       # 4-core mode: per-jt [KT, NK*JS] views

            def load(eng, kind, k0, nk, gi):
                dram, width, tl = (wc, JL, w_tiles) if kind == "w" \
                    else (xt, C, x_tiles)
                g = pool.tile([KT, nk * width], f32r, tag=f"g{gi}",
                              name=f"g{gi}")
                eng.dma_start(
                    out=g[:], in_=dram[:, k0 * width:(k0 + nk) * width])
                for kk in range(nk):
                    tl[k0 + kk] = g[:, kk * width:(kk + 1) * width]

            if FOUR_CORES:
                # X first on both queues (needed by every jt), then W
                # jt-major so per-jt epilogues pipeline with the W stream.
                # sync: Xk0-4 | Wjt0 | Wjt2 | Wjt4; scalar: Xk5-9 | Wjt1..5
                load(nc.sync, "x", 0, 5, 0)
                load(nc.scalar, "x", 5, 5, 1)
                KJ = NK * JS
                for jt in range(NJ):
                    g = pool.tile([KT, KJ], f32r, tag=f"wj{jt}",
                                  name=f"wj{jt}")
                    eng = nc.sync if jt % 2 == 0 else nc.scalar
                    eng.dma_start(out=g[:], in_=wc[:, jt * KJ:(jt + 1) * KJ])
                    wjt[jt] = g

                def w_slice(jt, k):
                    return wjt[jt][:, k * JS:(k + 1) * JS]
            else:
                # k-major on both queues with a fine-grained tail: the k9
                # chunks land last and are small, so only 3 matmuls plus the
                # epilogue remain after the final DMA byte.
                # sync:   Wk0-2 | Wk3-5 | Xk6-7 | Xk8 | Wk9
                # scalar: Xk0-2 | Xk3-5 | Wk6-8 | Xk9
                load(nc.sync, "w", 0, 3, 0)
                load(nc.scalar, "x", 0, 3, 1)
                load(nc.sync, "w", 3, 3, 2)
                load(nc.scalar, "x", 3, 3, 3)
                load(nc.sync, "x", 6, 2, 4)
                load(nc.scalar, "w", 6, 3, 5)
                load(nc.sync, "x", 8, 1, 6)
                load(nc.scalar, "x", 9, 1, 7)
                load(nc.sync, "w", 9, 1, 8)

                def w_slice(jt, k):
                    return w_tiles[k][:, jt * JS:(jt + 1) * JS]
            coef_all = pool.tile([JS, 3 * NJ * NL], f32, tag="coef")
            nc.gpsimd.dma_start(out=coef_all[:], in_=coef[:, :])

            ps = [pspool.tile([JS, C], f32, tag=f"ps{jt}", name=f"ps{jt}")
                  for jt in range(NJ)]
            if FOUR_CORES:
                mm_order = [(jt, k) for jt in range(NJ) for k in range(NK)]
            else:
                mm_order = [(jt, k) for k in range(NK) for jt in range(NJ)]
            for jt, k in mm_order:
                nc.tensor.matmul(
                    ps[jt][:], w_slice(jt, k), x_tiles[k],
                    start=(k == 0), stop=(k == NK - 1),
                )

            # Replicate the [JS, 12] coefficient vectors to [JS, 192] during
            # the DMA phase (DVE idle) so the combine ops run on flat APs.
            crep = pool.tile([JS, 3 * NJ * CA], f32, tag="crep")
            for i in range(3):
                for jt in range(NJ):
                    src = coef_all[:, i * NJ * NL + jt * NL:
                                   i * NJ * NL + (jt + 1) * NL]
                    dst = crep[:, (i * NJ + jt) * CA:(i * NJ + jt + 1) * CA]
                    nc.gpsimd.tensor_copy(
                        dst.rearrange("p (l b) -> p l b", b=BL),
                        src.broadcast_to([JS, NL, BL]))

            # Per-jt epilogue, pipelined: tanh on ACT, flat combine on DVE
            # (jt0, jt2) / GpSimd (jt1), per-jt output DMA.
            t_all = pool.tile([JS, NJ * C], f32, tag="t")
            res = pool.tile([JS, NJ * CA], f32, tag="res")
            tmp = pool.tile([JS, NJ * CA], f32, tag="tmp")
            tmp2 = pool.tile([JS, NJ * CA], f32, tag="tmp2")
            out3 = out.rearrange("(j p) c -> p j c", p=JS)
            for jt in range(NJ):
                # W was negated on the host, so psum = -(X @ W) already.
                nc.scalar.activation(
                    out=t_all[:, jt * C:(jt + 1) * C], in_=ps[jt][:],
                    func=mybir.ActivationFunctionType.Tanh,
                )
                t0 = jt * C
                tA = t_all[:, t0:t0 + CA]
                tU1 = t_all[:, t0 + CA + BL:t0 + CA + CU]
                tU0 = t_all[:, t0 + CA:t0 + CA + CA]
                rs = res[:, jt * CA:(jt + 1) * CA]
                ts = tmp[:, jt * CA:(jt + 1) * CA]
                ts2 = tmp2[:, jt * CA:(jt + 1) * CA]
                cof = [crep[:, (i * NJ + jt) * CA:(i * NJ + jt + 1) * CA]
                       for i in range(3)]
                ve = nc.gpsimd if jt in (1, 3) else nc.vector
                # three independent muls (pipeline on the engine), then adds
                ve.tensor_mul(rs, cof[0], tA)
                ve.tensor_mul(ts, cof[1], tU1)
                ve.tensor_mul(ts2, cof[2], tU0)
                ve.tensor_add(rs, rs, ts)
                ve.tensor_add(rs, rs, ts2)
                oeng = nc.sync if jt != 1 else nc.scalar
                oeng.dma_start(out=out3[:, jt, :], in_=rs)

    return nc


def _get_nc():
    global _cached
    if _cached is None:
        _cached = _build_nc()
        _cached.finalize()   # Bacc: runs reg alloc + codegen passes
    return _cached


def _host_coefs(alpha, fract, lambd, l):
    # All [12,...] fp32; compute in float64, cast at the end.
    a = alpha[:, 0].astype(np.float64)          # [12]
    f = fract[:, 0].astype(np.float64)          # [12]
    lam = lambd[:, 0, :, 0].astype(np.float64)  # [12, 200]
    ll = l[:, 0, :, 0].astype(np.float64)       # [12, 200]

    belta = np.zeros(NL)
    for la in range(NL):
        g_a1 = _gamma(a[la] + 1.0)
        belta[la] = sum(
            g_a1 / (_gamma(kk + 1.0) * _gamma(a[la] - kk + 1.0)) for kk in range(4)
        )
    cN = np.array([_gamma(a[la] + 1.0) / (6.0 * _gamma(a[la] - 2.0))
                   for la in range(NL)])

    # tile lambda/l from 200 -> 1200 (index n % 200)
    lam_t = np.tile(lam, (1, 6))                # [12, 1200]
    ll_t = np.tile(ll, (1, 6))                  # [12, 1200]

    inv_hf = (1.0 / H) ** f                     # 3**fract
    P = 2.0 * lam_t / belta[:, None] * inv_hf[:, None]
    Q = lam_t * ll_t / belta[:, None] / H
    R = Q * cN[:, None]
    return P.astype(np.float32), Q.astype(np.float32), R.astype(np.float32)


def _run_on_devices(nc, in_maps, device_ids):
    """run_bass_via_pjrt with an explicit device list (one core per SEngine
    pair) plus optional NTFF profiling. Returns a BassKernelResults."""
    import glob
    import os
    import tempfile

    import jax
    from jax.sharding import Mesh, PartitionSpec
    from jax.experimental.shard_map import shard_map

    import concourse.mybir as mybir
    from concourse.bass2jax import _bass_exec_p, install_neuronx_cc_hook
    from concourse.bass_utils import BassKernelResults, _process_ntff_profile

    install_neuronx_cc_hook()
    n_cores = len(device_ids)
    part_name = (nc.partition_id_tensor.name
                 if nc.partition_id_tensor else None)

    in_names, out_names, out_avals, zero_outs = [], [], [], []
    for alloc in nc.m.functions[0].allocations:
        if not isinstance(alloc, mybir.MemoryLocationSet):
            continue
        name = alloc.memorylocations[0].name
        if alloc.kind == "ExternalInput":
            if name != part_name:
                in_names.append(name)
        elif alloc.kind == "ExternalOutput":
            shape = tuple(alloc.tensor_shape)
            dtype = mybir.dt.np(alloc.dtype)
            out_names.append(name)
            out_avals.append(jax.core.ShapedArray(shape, dtype))
            zero_outs.append(np.zeros(shape, dtype))
    n_params = len(in_names)
    n_outs = len(out_avals)
    all_names = in_names + out_names
    if part_name is not None:
        all_names = all_names + [part_name]
    donate = tuple(range(n_params, n_params + n_outs))

    def _body(*args):
        operands = list(args)
        if part_name is not None:
            from concourse.bass2jax import partition_id_tensor
            operands.append(partition_id_tensor())
        outs = _bass_exec_p.bind(
            *operands,
            out_avals=tuple(out_avals),
            in_names=tuple(all_names),
            out_names=tuple(out_names),
            lowering_input_output_aliases=(),
            sim_require_finite=True,
            sim_require_nnan=True,
            nc=nc,
        )
        return tuple(outs)

    devices = [jax.devices()[i] for i in device_ids]
    mesh = Mesh(np.asarray(devices), ("core",))
    specs = (PartitionSpec("core"),) * (n_params + n_outs)
    sharded = jax.jit(
        shard_map(_body, mesh=mesh, in_specs=specs,
                  out_specs=(PartitionSpec("core"),) * n_outs,
                  check_rep=False),
        donate_argnums=donate, keep_unused=True,
    )
    concat_in = [
        np.concatenate([np.asarray(in_maps[c][nm]) for c in range(n_cores)],
                       axis=0) for nm in in_names
    ]
    concat_zeros = [
        np.zeros((n_cores * z.shape[0], *z.shape[1:]), z.dtype)
        for z in zero_outs
    ]

    trace = os.environ.get("BASS_TRACE") == "1"
    hook = None
    if trace:
        try:
            from antenv.axon_hooks import get_axon_ntff_profile_hook
            hook = get_axon_ntff_profile_hook()
        except ImportError:
            hook = None

    if hook is not None:
        neff_dir = tempfile.mkdtemp()
        with hook(neff_dir, [device_ids[0]]):
            out_arrs = sharded(*concat_in, *concat_zeros)
    else:
        out_arrs = sharded(*concat_in, *concat_zeros)

    results = [
        {nm: np.asarray(out_arrs[i]).reshape(n_cores, *out_avals[i].shape)[c]
         for i, nm in enumerate(out_names)}
        for c in range(n_cores)
    ]

    perf = BassKernelResults(results=results, instructions_and_trace=None,
                             profile_json=None, exec_time_ns=None)
    if hook is not None and glob.glob(os.path.join(neff_dir, "*_body*.ntff")):
        import gauge.profiler
        from concourse._compat import FishPath
        profile = gauge.profiler.Profile(
            profile_path=FishPath(neff_dir), kernel_dev_mode=True,
            profile_on_exit=False, bass_kernel=nc.m,
            offline_processing=True, fname="*_body*",
            metadata={"artifacts_path": neff_dir},
        )
        p = _process_ntff_profile(
            profile, neff_dir, nc, device_ids, [device_ids[0]], False, {},
            trace_events=False)
        perf = p.as_bass_kernel_results(results)
    return perf


def kernel(A, WW, train_init, alpha, fract, lambd, l, A_y_list):
    from concourse.bass_utils import run_bass_kernel_spmd

    if USE_BF16:
        import ml_dtypes
        mm_dt = ml_dtypes.bfloat16
    else:
        mm_dt = np.float32

    A = np.asarray(A, dtype=np.float32)
    WW = np.asarray(WW, dtype=np.float32)
    train_init = np.asarray(train_init, dtype=np.float32)

    P, Q, R = _host_coefs(
        np.asarray(alpha, np.float32), np.asarray(fract, np.float32),
        np.asarray(lambd, np.float32), np.asarray(l, np.float32))

    Wneg = -WW[:, :, 0]                         # [1200, 1200]

    xts, wcs, coefs = {}, {}, {}
    for beta in range(PB):
        bsl = slice(beta * BL, (beta + 1) * BL)
        xa = A[:, bsl, :, 0].transpose(2, 0, 1).reshape(N, CA)      # col=la*BL+b
        xu = train_init[bsl, :, :, 1].transpose(1, 2, 0).reshape(N, CU)  # col=t*BL+b
        XT = np.concatenate([xa, xu], axis=1)                       # [1200, 400]
        # partition-major: [KT, NK*C], col = k*C + c
        xts[beta] = np.ascontiguousarray(
            XT.reshape(NK, KT, C).transpose(1, 0, 2).reshape(KT, NK * C),
            dtype=mm_dt)
    for g in range(PJ):
        gsl = slice(g * JL, (g + 1) * JL)
        if FOUR_CORES:
            # partition-major, jt-major: col = jt*NK*JS + k*JS + s
            wcs[g] = np.ascontiguousarray(
                Wneg[:, gsl].reshape(NK, KT, NJ, JS).transpose(1, 2, 0, 3)
                .reshape(KT, NK * JL), dtype=mm_dt)
        else:
            # partition-major, k-major: col = k*JL + j
            wcs[g] = np.ascontiguousarray(
                Wneg[:, gsl].reshape(NK, KT, JL).transpose(1, 0, 2)
                .reshape(KT, NK * JL), dtype=mm_dt)
        # coef [JS, 108]: col = kind*36 + jt*12 + la
        kinds = [M[:, gsl].reshape(NL, NJ, JS).transpose(2, 1, 0)
                 for M in (P, Q, R)]                                # [100, 3, 12]
        coefs[g] = np.ascontiguousarray(
            np.stack(kinds, axis=1).reshape(JS, 3 * NJ * NL), dtype=np.float32)

    in_maps = []
    for core in range(PB * PJ):
        beta, g = divmod(core, PJ)
        in_maps.append({"xt": xts[beta], "wc": wcs[g], "coef": coefs[g]})

    nc = _get_nc()
    if FOUR_CORES:
        res = _run_on_devices(nc, in_maps, DEVICE_IDS)
    else:
        res = run_bass_kernel_spmd(nc, in_maps, core_ids=list(range(PB * PJ)))
    kernel.last_results = res

    full = np.empty((B, NL, N), dtype=np.float32)
    for core in range(PB * PJ):
        beta, g = divmod(core, PJ)
        o = res.results[core]["out"]            # [300, 192], col = la*BL+b
        full[beta * BL:(beta + 1) * BL, :, g * JL:(g + 1) * JL] = (
            o.reshape(JL, NL, BL).transpose(2, 1, 0))
    return full.reshape(B, NL, N, 1)


# revision 35
# speedup vs baseline: 1.2332x; 1.2332x over previous
# Trainium2 Bass kernel for nn_FCM_series_1 (gnn_message_passing).
#
# Math (derived from the reference):
#   aggregate(X, WW)[l,b,j] = tanh(-sum_i X[l,b,i] * WW[i,j])
#   T_A  = aggregate(A, WW)                     (12 lags x B rows)
#   U[t] = aggregate(train_init[:,:,t,1], WW)   (13 unique rows per batch;
#          A_N_OLD[la] = U[la], A_0_NEW[la] = U[la+1])
#   out[b,la,j] = P[la,j]*T_A[la,b,j] + Q[la,j]*U[la+1,b,j] + R[la,j]*U[la,b,j]
# with host-computable coefficients
#   P[la,j] = 2 * lambd[la, j%200] / belta[la] * 3**fract[la]
#   Q[la,j] = 3 * lambd[la, j%200] * l[la, j%200] / belta[la]
#   R[la,j] = Q[la,j] * Gamma(a+1)/(6*Gamma(a-2))
#   belta[la] = sum_{k=0..3} Gamma(a+1)/(Gamma(k+1)*Gamma(a-k+1))
#
# Sharding over 8 cores: batch split x2 (16 each), output node dim j split x4
# (300 each). Per core one matmul chain: lhsT=W-chunk tiles, rhs=X^T tiles,
# PSUM-accumulated over 10 k-tiles of 120, in float32r (single-pass fp32 PE
# mode, 4x faster than fp32 LOW_HIGH). W is negated on the host so psum
# already holds -X@W; tanh on ScalarE; coefficient combine on VectorE with
# 0-stride broadcast APs; per-core [300,192] result re-assembled on the host.
#
# HBM layouts are host-repacked to partition-major so every DMA descriptor is
# one large contiguous run per partition; input DMAs are split between the two
# HWDGE queues (sync for W, scalar for X) to double aggregate DMA throughput.

import math

import numpy as np

LAG = 13
B = 32
N = 1200
H = 1.0 / 3.0

# FOUR_CORES: run on one NeuronCore per SEngine pair (devices 0,2,4,6).
# Measured WORSE (46.5 us vs 37.3): per-core DMA is capped ~215 GB/s even
# with the paired core idle, so fewer cores just means more bytes per core.
FOUR_CORES = False
PB = 2          # batch shards
PJ = 2 if FOUR_CORES else 4   # j shards
BL = B // PB    # 16 batches per core
JL = N // PJ    # output nodes per core (600 / 300)
NL = LAG - 1    # 12
CA = NL * BL    # 192 cols: T_A block, col = la*BL + b
CU = LAG * BL   # 208 cols: U block,  col = CA + t*BL + b
C = CA + CU     # 400 matmul moving cols
KT = 120        # contraction tile
NK = N // KT    # 10
JS = 100        # j subtile (psum partition dim)
NJ = JL // JS   # j subtiles per core (6 / 3)
NCH = 2         # input DMA chunks per tensor (5 k-tiles each)
USE_BF16 = False  # matmul operand dtype: bf16 halves DMA bytes, ~2.5e-3 rel err
DEVICE_IDS = [0, 2, 4, 6] if FOUR_CORES else list(range(8))

_cached = None


def _gamma(x):
    return math.gamma(x)


def _build_nc():
    import concourse.bacc as bacc
    import concourse.mybir as mybir
    from concourse.tile import TileContext

    f32 = mybir.dt.float32
    f32r = mybir.dt.bfloat16 if USE_BF16 else mybir.dt.float32r
    nc = bacc.Bacc(None, target_bir_lowering=False)

    # partition-major repacked inputs (see kernel() for layouts)
    xt = nc.dram_tensor("xt", [KT, NK * C], f32r, kind="ExternalInput")
    wc = nc.dram_tensor("wc", [KT, NK * JL], f32r, kind="ExternalInput")
    coef = nc.dram_tensor("coef", [JS, 3 * NJ * NL], f32, kind="ExternalInput")
    out = nc.dram_tensor("out", [JL, CA], f32, kind="ExternalOutput")

    with TileContext(nc) as tc:
        with (
            tc.tile_pool(name="sb", bufs=1) as pool,
            tc.tile_pool(name="ps", bufs=1, space="PSUM") as pspool,
        ):
            x_tiles = [None] * NK   # per-k [KT, C] views
            w_tiles = [None] * NK   # 8-core mode: per-k [KT, JL] views
            wjt = [None] * NJ       # 4-core mode: per-jt [KT, NK*JS] views

            def load(eng, kind, k0, nk, gi):
                dram, width, tl = (wc, JL, w_tiles) if kind == "w" \
                    else (xt, C, x_tiles)
                g = pool.tile([KT, nk * width], f32r, tag=f"g{gi}",
                              name=f"g{gi}")
                eng.dma_start(
                    out=g[:], in_=dram[:, k0 * width:(k0 + nk) * width])
                for kk in range(nk):
                    tl[k0 + kk] = g[:, kk * width:(kk + 1) * width]

            if FOUR_CORES:
                # X first on both queues (needed by every jt), then W
                # jt-major so per-jt epilogues pipeline with the W stream.
                # sync: Xk0-4 | Wjt0 | Wjt2 | Wjt4; scalar: Xk5-9 | Wjt1..5
                load(nc.sync, "x", 0, 5, 0)
                load(nc.scalar, "x", 5, 5, 1)
                KJ = NK * JS
                for jt in range(NJ):
                    g = pool.tile([KT, KJ], f32r, tag=f"wj{jt}",
                                  name=f"wj{jt}")
                    eng = nc.sync if jt % 2 == 0 else nc.scalar
                    eng.dma_start(out=g[:], in_=wc[:, jt * KJ:(jt + 1) * KJ])
                    wjt[jt] = g

                def w_slice(jt, k):
                    return wjt[jt][:, k * JS:(k + 1) * JS]
            else:
                # k-major on both queues with a fine-grained tail: the k9
                # chunks land last and are small, so only 3 matmuls plus the
                # epilogue remain after the final DMA byte.
                # sync:   Wk0-2 | Wk3-5 | Xk6-7 | Xk8 | Wk9
                # scalar: Xk0-2 | Xk3-5 | Wk6-8 | Xk9
                load(nc.sync, "w", 0, 3, 0)
                load(nc.scalar, "x", 0, 3, 1)
                load(nc.sync, "w", 3, 3, 2)
                load(nc.scalar, "x", 3, 3, 3)
                load(nc.sync, "x", 6, 2, 4)
                load(nc.scalar, "w", 6, 3, 5)
                load(nc.sync, "x", 8, 1, 6)
                load(nc.scalar, "x", 9, 1, 7)
                load(nc.sync, "w", 9, 1, 8)

                def w_slice(jt, k):
                    return w_tiles[k][:, jt * JS:(jt + 1) * JS]
            coef_all = pool.tile([JS, 3 * NJ * NL], f32, tag="coef")
            nc.gpsimd.dma_start(out=coef_all[:], in_=coef[:, :])

            ps = [pspool.tile([JS, C], f32, tag=f"ps{jt}", name=f"ps{jt}")
                  for jt in range(NJ)]
            if FOUR_CORES:
                mm_order = [(jt, k) for jt in range(NJ) for k in range(NK)]
            else:
                mm_order = [(jt, k) for k in range(NK) for jt in range(NJ)]
            for jt, k in mm_order:
                nc.tensor.matmul(
                    ps[jt][:], w_slice(jt, k), x_tiles[k],
                    start=(k == 0), stop=(k == NK - 1),
                )

            # Replicate the [JS, 12] coefficient vectors to [JS, 192] during
            # the DMA phase (DVE idle) so the combine ops run on flat APs.
            crep = pool.tile([JS, 3 * NJ * CA], f32, tag="crep")
            for i in range(3):
                for jt in range(NJ):
                    src = coef_all[:, i * NJ * NL + jt * NL:
                                   i * NJ * NL + (jt + 1) * NL]
                    dst = crep[:, (i * NJ + jt) * CA:(i * NJ + jt + 1) * CA]
                    nc.gpsimd.tensor_copy(
                        dst.rearrange("p (l b) -> p l b", b=BL),
                        src.broadcast_to([JS, NL, BL]))

            # Per-jt epilogue, pipelined: tanh on ACT, flat combine on DVE
            # (jt0, jt2) / GpSimd (jt1), per-jt output DMA.
            t_all = pool.tile([JS, NJ * C], f32, tag="t")
            res = pool.tile([JS, NJ * CA], f32, tag="res")
            tmp = pool.tile([JS, NJ * CA], f32, tag="tmp")
            tmp2 = pool.tile([JS, NJ * CA], f32, tag="tmp2")
            out3 = out.rearrange("(j p) c -> p j c", p=JS)
            for jt in range(NJ):
                # W was negated on the host, so psum = -(X @ W) already.
                nc.scalar.activation(
                    out=t_all[:, jt * C:(jt + 1) * C], in_=ps[jt][:],
                    func=mybir.ActivationFunctionType.Tanh,
                )
                t0 = jt * C
                tA = t_all[:, t0:t0 + CA]
                tU1 = t_all[:, t0 + CA + BL:t0 + CA + CU]
                tU0 = t_all[:, t0 + CA:t0 + CA + CA]
                rs = res[:, jt * CA:(jt + 1) * CA]
                ts = tmp[:, jt * CA:(jt + 1) * CA]
                ts2 = tmp2[:, jt * CA:(jt + 1) * CA]
                cof = [crep[:, (i * NJ + jt) * CA:(i * NJ + jt + 1) * CA]
                       for i in range(3)]
                ve = nc.gpsimd if jt in (1, 3) else nc.vector
                # three independent muls (pipeline on the engine), then adds
                ve.tensor_mul(rs, cof[0], tA)
                ve.tensor_mul(ts, cof[1], tU1)
                ve.tensor_mul(ts2, cof[2], tU0)
                ve.tensor_add(rs, rs, ts)
                ve.tensor_add(rs, rs, ts2)
                oeng = nc.sync if jt != 1 else nc.scalar
                oeng.dma_start(out=out3[:, jt, :], in_=rs)

    return nc


def _get_nc():
    global _cached
    if _cached is None:
        _cached = _build_nc()
        _cached.finalize()   # Bacc: runs reg alloc + codegen passes
    return _cached


def _host_coefs(alpha, fract, lambd, l):
    # All [12,...] fp32; compute in float64, cast at the end.
    a = alpha[:, 0].astype(np.float64)          # [12]
    f = fract[:, 0].astype(np.float64)          # [12]
    lam = lambd[:, 0, :, 0].astype(np.float64)  # [12, 200]
    ll = l[:, 0, :, 0].astype(np.float64)       # [12, 200]

    belta = np.zeros(NL)
    for la in range(NL):
        g_a1 = _gamma(a[la] + 1.0)
        belta[la] = sum(
            g_a1 / (_gamma(kk + 1.0) * _gamma(a[la] - kk + 1.0)) for kk in range(4)
        )
    cN = np.array([_gamma(a[la] + 1.0) / (6.0 * _gamma(a[la] - 2.0))
                   for la in range(NL)])

    # tile lambda/l from 200 -> 1200 (index n % 200)
    lam_t = np.tile(lam, (1, 6))                # [12, 1200]
    ll_t = np.tile(ll, (1, 6))                  # [12, 1200]

    inv_hf = (1.0 / H) ** f                     # 3**fract
    P = 2.0 * lam_t / belta[:, None] * inv_hf[:, None]
    Q = lam_t * ll_t / belta[:, None] / H
    R = Q * cN[:, None]
    return P.astype(np.float32), Q.astype(np.float32), R.astype(np.float32)


def _run_on_devices(nc, in_maps, device_ids):
    """run_bass_via_pjrt with an explicit device list (one core per SEngine
    pair) plus optional NTFF profiling. Returns a BassKernelResults."""
    import glob
    import os
    import tempfile

    import jax
    from jax.sharding import Mesh, PartitionSpec
    from jax.experimental.shard_map import shard_map

    import concourse.mybir as mybir
    from concourse.bass2jax import _bass_exec_p, install_neuronx_cc_hook
    from concourse.bass_utils import BassKernelResults, _process_ntff_profile

    install_neuronx_cc_hook()
    n_cores = len(device_ids)
    part_name = (nc.partition_id_tensor.name
                 if nc.partition_id_tensor else None)

    in_names, out_names, out_avals, zero_outs = [], [], [], []
    for alloc in nc.m.functions[0].allocations:
        if not isinstance(alloc, mybir.MemoryLocationSet):
            continue
        name = alloc.memorylocations[0].name
        if alloc.kind == "ExternalInput":
            if name != part_name:
                in_names.append(name)
        elif alloc.kind == "ExternalOutput":
            shape = tuple(alloc.tensor_shape)
            dtype = mybir.dt.np(alloc.dtype)
            out_names.append(name)
            out_avals.append(jax.core.ShapedArray(shape, dtype))
            zero_outs.append(np.zeros(shape, dtype))
    n_params = len(in_names)
    n_outs = len(out_avals)
    all_names = in_names + out_names
    if part_name is not None:
        all_names = all_names + [part_name]
    donate = tuple(range(n_params, n_params + n_outs))

    def _body(*args):
        operands = list(args)
        if part_name is not None:
            from concourse.bass2jax import partition_id_tensor
            operands.append(partition_id_tensor())
        outs = _bass_exec_p.bind(
            *operands,
            out_avals=tuple(out_avals),
            in_names=tuple(all_names),
            out_names=tuple(out_names),
            lowering_input_output_aliases=(),
            sim_require_finite=True,
            sim_require_nnan=True,
            nc=nc,
        )
        return tuple(outs)

    devices = [jax.devices()[i] for i in device_ids]
    mesh = Mesh(np.asarray(devices), ("core",))
    specs = (PartitionSpec("core"),) * (n_params + n_outs)
    sharded = jax.jit(
        shard_map(_body, mesh=mesh, in_specs=specs,
                  out_specs=(PartitionSpec("core"),) * n_outs,
                  check_rep=False),
        donate_argnums=donate, keep_unused=True,
    )
    concat_in = [
        np.concatenate([np.asarray(in_maps[c][nm]) for c in range(n_cores)],
                       axis=0) for nm in in_names
    ]
    concat_zeros = [
        np.zeros((n_cores * z.shape[0], *z.shape[1:]), z.dtype)
        for z in zero_outs
    ]

    trace = os.environ.get("BASS_TRACE") == "1"
    hook = None
    if trace:
        try:
            from antenv.axon_hooks import get_axon_ntff_profile_hook
            hook = get_axon_ntff_profile_hook()
        except ImportError:
            hook = None

    if hook is not None:
        neff_dir = tempfile.mkdtemp()
        with hook(neff_dir, [device_ids[0]]):
            out_arrs = sharded(*concat_in, *concat_zeros)
    else:
        out_arrs = sharded(*concat_in, *concat_zeros)

    results = [
        {nm: np.asarray(out_arrs[i]).reshape(n_cores, *out_avals[i].shape)[c]
         for i, nm in enumerate(out_names)}
        for c in range(n_cores)
    ]

    perf = BassKernelResults(results=results, instructions_and_trace=None,
                             profile_json=None, exec_time_ns=None)
    if hook is not None and glob.glob(os.path.join(neff_dir, "*_body*.ntff")):
        import gauge.profiler
        from concourse._compat import FishPath
        profile = gauge.profiler.Profile(
            profile_path=FishPath(neff_dir), kernel_dev_mode=True,
            profile_on_exit=False, bass_kernel=nc.m,
            offline_processing=True, fname="*_body*",
            metadata={"artifacts_path": neff_dir},
        )
        p = _process_ntff_profile(
            profile, neff_dir, nc, device_ids, [device_ids[0]], False, {},
            trace_events=False)
        perf = p.as_bass_kernel_results(results)
    return perf


def kernel(A, WW, train_init, alpha, fract, lambd, l, A_y_list):
    from concourse.bass_utils import run_bass_kernel_spmd

    if USE_BF16:
        import ml_dtypes
        mm_dt = ml_dtypes.bfloat16
    else:
        mm_dt = np.float32

    A = np.asarray(A, dtype=np.float32)
    WW = np.asarray(WW, dtype=np.float32)
    train_init = np.asarray(train_init, dtype=np.float32)

    P, Q, R = _host_coefs(
        np.asarray(alpha, np.float32), np.asarray(fract, np.float32),
        np.asarray(lambd, np.float32), np.asarray(l, np.float32))

    Wneg = -WW[:, :, 0]                         # [1200, 1200]

    xts, wcs, coefs = {}, {}, {}
    for beta in range(PB):
        bsl = slice(beta * BL, (beta + 1) * BL)
        xa = A[:, bsl, :, 0].transpose(2, 0, 1).reshape(N, CA)      # col=la*BL+b
        xu = train_init[bsl, :, :, 1].transpose(1, 2, 0).reshape(N, CU)  # col=t*BL+b
        XT = np.concatenate([xa, xu], axis=1)                       # [1200, 400]
        # partition-major: [KT, NK*C], col = k*C + c
        xts[beta] = np.ascontiguousarray(
            XT.reshape(NK, KT, C).transpose(1, 0, 2).reshape(KT, NK * C),
            dtype=mm_dt)
    for g in range(PJ):
        gsl = slice(g * JL, (g + 1) * JL)
        if FOUR_CORES:
            # partition-major, jt-major: col = jt*NK*JS + k*JS + s
            wcs[g] = np.ascontiguousarray(
                Wneg[:, gsl].reshape(NK, KT, NJ, JS).transpose(1, 2, 0, 3)
                .reshape(KT, NK * JL), dtype=mm_dt)
        else:
            # partition-major, k-major: col = k*JL + j
            wcs[g] = np.ascontiguousarray(
                Wneg[:, gsl].reshape(NK, KT, JL).transpose(1, 0, 2)
                .reshape(KT, NK * JL), dtype=mm_dt)
        # coef [JS, 108]: col = kind*36 + jt*12 + la
        kinds = [M[:, gsl].reshape(NL, NJ, JS).transpose(2, 1, 0)
                 for M in (P, Q, R)]                                # [100, 3, 12]
        coefs[g] = np.ascontiguousarray(
            np.stack(kinds, axis=1).reshape(JS, 3 * NJ * NL), dtype=np.float32)

    in_maps = []
    for core in range(PB * PJ):
        beta, g = divmod(core, PJ)
        in_maps.append({"xt": xts[beta], "wc": wcs[g], "coef": coefs[g]})

    nc = _get_nc()
    if FOUR_CORES:
        res = _run_on_devices(nc, in_maps, DEVICE_IDS)
    else:
        res = run_bass_kernel_spmd(nc, in_maps, core_ids=list(range(PB * PJ)))
    kernel.last_results = res

    full = np.empty((B, NL, N), dtype=np.float32)
    for core in range(PB * PJ):
        beta, g = divmod(core, PJ)
        o = res.results[core]["out"]            # [300, 192], col = la*BL+b
        full[beta * BL:(beta + 1) * BL, :, g * JL:(g + 1) * JL] = (
            o.reshape(JL, NL, BL).transpose(2, 1, 0))
    return full.reshape(B, NL, N, 1)


# revision 38
# speedup vs baseline: 1.2629x; 1.0240x over previous
# Trainium2 Bass kernel for nn_FCM_series_1 (gnn_message_passing).
#
# Math (derived from the reference):
#   aggregate(X, WW)[l,b,j] = tanh(-sum_i X[l,b,i] * WW[i,j])
#   T_A  = aggregate(A, WW)                     (12 lags x B rows)
#   U[t] = aggregate(train_init[:,:,t,1], WW)   (13 unique rows per batch;
#          A_N_OLD[la] = U[la], A_0_NEW[la] = U[la+1])
#   out[b,la,j] = P[la,j]*T_A[la,b,j] + Q[la,j]*U[la+1,b,j] + R[la,j]*U[la,b,j]
# with host-computable coefficients
#   P[la,j] = 2 * lambd[la, j%200] / belta[la] * 3**fract[la]
#   Q[la,j] = 3 * lambd[la, j%200] * l[la, j%200] / belta[la]
#   R[la,j] = Q[la,j] * Gamma(a+1)/(6*Gamma(a-2))
#   belta[la] = sum_{k=0..3} Gamma(a+1)/(Gamma(k+1)*Gamma(a-k+1))
#
# Sharding over 8 cores: batch split x2 (16 each), output node dim j split x4
# (300 each). Per core one matmul chain: lhsT=W-chunk tiles, rhs=X^T tiles,
# PSUM-accumulated over 10 k-tiles of 120, in float32r (single-pass fp32 PE
# mode, 4x faster than fp32 LOW_HIGH). W is negated on the host so psum
# already holds -X@W; tanh on ScalarE; coefficient combine on VectorE with
# 0-stride broadcast APs; per-core [300,192] result re-assembled on the host.
#
# HBM layouts are host-repacked to partition-major so every DMA descriptor is
# one large contiguous run per partition; input DMAs are split between the two
# HWDGE queues (sync for W, scalar for X) to double aggregate DMA throughput.

import math

import numpy as np

LAG = 13
B = 32
N = 1200
H = 1.0 / 3.0

# FOUR_CORES: run on one NeuronCore per SEngine pair (devices 0,2,4,6).
# Measured WORSE (46.5 us vs 37.3): per-core DMA is capped ~215 GB/s even
# with the paired core idle, so fewer cores just means more bytes per core.
FOUR_CORES = False
PB = 2          # batch shards
PJ = 2 if FOUR_CORES else 4   # j shards
BL = B // PB    # 16 batches per core
JL = N // PJ    # output nodes per core (600 / 300)
NL = LAG - 1    # 12
CA = NL * BL    # 192 cols: T_A block, col = la*BL + b
CU = LAG * BL   # 208 cols: U block,  col = CA + t*BL + b
C = CA + CU     # 400 matmul moving cols
KT = 120        # contraction tile
NK = N // KT    # 10
JS = 100        # j subtile (psum partition dim)
NJ = JL // JS   # j subtiles per core (6 / 3)
NCH = 2         # input DMA chunks per tensor (5 k-tiles each)
USE_BF16 = False  # matmul operand dtype: bf16 halves DMA bytes, ~2.5e-3 rel err
DEVICE_IDS = [0, 2, 4, 6] if FOUR_CORES else list(range(8))

_cached = None


def _gamma(x):
    return math.gamma(x)


def _build_nc():
    import concourse.bacc as bacc
    import concourse.mybir as mybir
    from concourse.tile import TileContext

    f32 = mybir.dt.float32
    f32r = mybir.dt.bfloat16 if USE_BF16 else mybir.dt.float32r
    nc = bacc.Bacc(None, target_bir_lowering=False)

    # partition-major repacked inputs (see kernel() for layouts)
    xt = nc.dram_tensor("xt", [KT, NK * C], f32r, kind="ExternalInput")
    wc = nc.dram_tensor("wc", [KT, NK * JL], f32r, kind="ExternalInput")
    coef = nc.dram_tensor("coef", [JS, 3 * NJ * NL], f32, kind="ExternalInput")
    out = nc.dram_tensor("out", [JL, CA], f32, kind="ExternalOutput")

    with TileContext(nc) as tc:
        with (
            tc.tile_pool(name="sb", bufs=1) as pool,
            tc.tile_pool(name="ps", bufs=1, space="PSUM") as pspool,
        ):
            x_tiles = [None] * NK   # per-k [KT, C] views
            w_tiles = [None] * NK   # 8-core mode: per-k [KT, JL] views
            wjt = [None] * NJ       # 4-core mode: per-jt [KT, NK*JS] views

            def load(eng, kind, k0, nk, gi):
                dram, width, tl = (wc, JL, w_tiles) if kind == "w" \
                    else (xt, C, x_tiles)
                g = pool.tile([KT, nk * width], f32r, tag=f"g{gi}",
                              name=f"g{gi}")
                eng.dma_start(
                    out=g[:], in_=dram[:, k0 * width:(k0 + nk) * width])
                for kk in range(nk):
                    tl[k0 + kk] = g[:, kk * width:(kk + 1) * width]

            if FOUR_CORES:
                # X first on both queues (needed by every jt), then W
                # jt-major so per-jt epilogues pipeline with the W stream.
                # sync: Xk0-4 | Wjt0 | Wjt2 | Wjt4; scalar: Xk5-9 | Wjt1..5
                load(nc.sync, "x", 0, 5, 0)
                load(nc.scalar, "x", 5, 5, 1)
                KJ = NK * JS
                for jt in range(NJ):
                    g = pool.tile([KT, KJ], f32r, tag=f"wj{jt}",
                                  name=f"wj{jt}")
                    eng = nc.sync if jt % 2 == 0 else nc.scalar
                    eng.dma_start(out=g[:], in_=wc[:, jt * KJ:(jt + 1) * KJ])
                    wjt[jt] = g

                def w_slice(jt, k):
                    return wjt[jt][:, k * JS:(k + 1) * JS]
            else:
                # k-major on both queues with a fine-grained tail: the k9
                # chunks land last and are small, so only 3 matmuls plus the
                # epilogue remain after the final DMA byte.
                # sync:   Wk0-2 | Wk3-5 | Xk6-7 | Xk8 | Wk9
                # scalar: Xk0-2 | Xk3-5 | Wk6-8 | Xk9
                load(nc.sync, "w", 0, 3, 0)
                load(nc.scalar, "x", 0, 3, 1)
                load(nc.sync, "w", 3, 3, 2)
                load(nc.scalar, "x", 3, 3, 3)
                load(nc.sync, "x", 6, 2, 4)
                load(nc.scalar, "w", 6, 3, 5)
                load(nc.sync, "x", 8, 1, 6)
                load(nc.scalar, "x", 9, 1, 7)
                load(nc.sync, "w", 9, 1, 8)

                def w_slice(jt, k):
                    return w_tiles[k][:, jt * JS:(jt + 1) * JS]
            coef_all = pool.tile([JS, 3 * NJ * NL], f32, tag="coef")
            nc.gpsimd.dma_start(out=coef_all[:], in_=coef[:, :])

            # Warm up the PE clock gate (HAM) with throwaway bf16 matmuls
            # while inputs stream: otherwise the ~22 matmuls left after the
            # last DMA chunk run at the cold 333 ns cadence instead of 180.
            bf16 = mybir.dt.bfloat16
            scratch = pool.tile([KT, C], bf16, tag="scr")
            nc.vector.memset(scratch[:], 0)
            psw = pspool.tile([JS, C], f32, tag="psw", name="psw")
            for i in range(36):
                nc.tensor.matmul(psw[:], scratch[:, 0:JS], scratch[:],
                                 start=True, stop=True)

            ps = [pspool.tile([JS, C], f32, tag=f"ps{jt}", name=f"ps{jt}")
                  for jt in range(NJ)]
            if FOUR_CORES:
                mm_order = [(jt, k) for jt in range(NJ) for k in range(NK)]
            else:
                mm_order = [(jt, k) for k in range(NK) for jt in range(NJ)]
            for jt, k in mm_order:
                nc.tensor.matmul(
                    ps[jt][:], w_slice(jt, k), x_tiles[k],
                    start=(k == 0), stop=(k == NK - 1),
                )

            # Replicate the [JS, 12] coefficient vectors to [JS, 192] during
            # the DMA phase (DVE idle) so the combine ops run on flat APs.
            crep = pool.tile([JS, 3 * NJ * CA], f32, tag="crep")
            for i in range(3):
                for jt in range(NJ):
                    src = coef_all[:, i * NJ * NL + jt * NL:
                                   i * NJ * NL + (jt + 1) * NL]
                    dst = crep[:, (i * NJ + jt) * CA:(i * NJ + jt + 1) * CA]
                    # DVE is idle during the DMA phase; GpSimd would contend
                    # for the shared DVE/GpSimd SBUF port later.
                    nc.vector.tensor_copy(
                        dst.rearrange("p (l b) -> p l b", b=BL),
                        src.broadcast_to([JS, NL, BL]))

            # Per-jt epilogue, pipelined: tanh on ACT, flat combine on DVE
            # (jt0, jt2) / GpSimd (jt1), per-jt output DMA.
            t_all = pool.tile([JS, NJ * C], f32, tag="t")
            res = pool.tile([JS, NJ * CA], f32, tag="res")
            tmp = pool.tile([JS, NJ * CA], f32, tag="tmp")
            tmp2 = pool.tile([JS, NJ * CA], f32, tag="tmp2")
            out3 = out.rearrange("(j p) c -> p j c", p=JS)
            for jt in range(NJ):
                # W was negated on the host, so psum = -(X @ W) already.
                nc.scalar.activation(
                    out=t_all[:, jt * C:(jt + 1) * C], in_=ps[jt][:],
                    func=mybir.ActivationFunctionType.Tanh,
                )
                t0 = jt * C
                tA = t_all[:, t0:t0 + CA]
                tU1 = t_all[:, t0 + CA + BL:t0 + CA + CU]
                tU0 = t_all[:, t0 + CA:t0 + CA + CA]
                rs = res[:, jt * CA:(jt + 1) * CA]
                ts = tmp[:, jt * CA:(jt + 1) * CA]
                ts2 = tmp2[:, jt * CA:(jt + 1) * CA]
                cof = [crep[:, (i * NJ + jt) * CA:(i * NJ + jt + 1) * CA]
                       for i in range(3)]
                # All combines on DVE: GpSimd shares the DVE SBUF port pair,
                # so "parallel" gp combines just halve both engines' rates.
                ve = nc.vector
                # three independent muls (pipeline on the engine), then adds
                ve.tensor_mul(rs, cof[0], tA)
                ve.tensor_mul(ts, cof[1], tU1)
                ve.tensor_mul(ts2, cof[2], tU0)
                ve.tensor_add(rs, rs, ts)
                ve.tensor_add(rs, rs, ts2)
                oeng = nc.sync if jt != 1 else nc.scalar
                oeng.dma_start(out=out3[:, jt, :], in_=rs)

    return nc


def _get_nc():
    global _cached
    if _cached is None:
        _cached = _build_nc()
        _cached.finalize()   # Bacc: runs reg alloc + codegen passes
    return _cached


def _host_coefs(alpha, fract, lambd, l):
    # All [12,...] fp32; compute in float64, cast at the end.
    a = alpha[:, 0].astype(np.float64)          # [12]
    f = fract[:, 0].astype(np.float64)          # [12]
    lam = lambd[:, 0, :, 0].astype(np.float64)  # [12, 200]
    ll = l[:, 0, :, 0].astype(np.float64)       # [12, 200]

    belta = np.zeros(NL)
    for la in range(NL):
        g_a1 = _gamma(a[la] + 1.0)
        belta[la] = sum(
            g_a1 / (_gamma(kk + 1.0) * _gamma(a[la] - kk + 1.0)) for kk in range(4)
        )
    cN = np.array([_gamma(a[la] + 1.0) / (6.0 * _gamma(a[la] - 2.0))
                   for la in range(NL)])

    # tile lambda/l from 200 -> 1200 (index n % 200)
    lam_t = np.tile(lam, (1, 6))                # [12, 1200]
    ll_t = np.tile(ll, (1, 6))                  # [12, 1200]

    inv_hf = (1.0 / H) ** f                     # 3**fract
    P = 2.0 * lam_t / belta[:, None] * inv_hf[:, None]
    Q = lam_t * ll_t / belta[:, None] / H
    R = Q * cN[:, None]
    return P.astype(np.float32), Q.astype(np.float32), R.astype(np.float32)


def _run_on_devices(nc, in_maps, device_ids):
    """run_bass_via_pjrt with an explicit device list (one core per SEngine
    pair) plus optional NTFF profiling. Returns a BassKernelResults."""
    import glob
    import os
    import tempfile

    import jax
    from jax.sharding import Mesh, PartitionSpec
    from jax.experimental.shard_map import shard_map

    import concourse.mybir as mybir
    from concourse.bass2jax import _bass_exec_p, install_neuronx_cc_hook
    from concourse.bass_utils import BassKernelResults, _process_ntff_profile

    install_neuronx_cc_hook()
    n_cores = len(device_ids)
    part_name = (nc.partition_id_tensor.name
                 if nc.partition_id_tensor else None)

    in_names, out_names, out_avals, zero_outs = [], [], [], []
    for alloc in nc.m.functions[0].allocations:
        if not isinstance(alloc, mybir.MemoryLocationSet):
            continue
        name = alloc.memorylocations[0].name
        if alloc.kind == "ExternalInput":
            if name != part_name:
                in_names.append(name)
        elif alloc.kind == "ExternalOutput":
            shape = tuple(alloc.tensor_shape)
            dtype = mybir.dt.np(alloc.dtype)
            out_names.append(name)
            out_avals.append(jax.core.ShapedArray(shape, dtype))
            zero_outs.append(np.zeros(shape, dtype))
    n_params = len(in_names)
    n_outs = len(out_avals)
    all_names = in_names + out_names
    if part_name is not None:
        all_names = all_names + [part_name]
    donate = tuple(range(n_params, n_params + n_outs))

    def _body(*args):
        operands = list(args)
        if part_name is not None:
            from concourse.bass2jax import partition_id_tensor
            operands.append(partition_id_tensor())
        outs = _bass_exec_p.bind(
            *operands,
            out_avals=tuple(out_avals),
            in_names=tuple(all_names),
            out_names=tuple(out_names),
            lowering_input_output_aliases=(),
            sim_require_finite=True,
            sim_require_nnan=True,
            nc=nc,
        )
        return tuple(outs)

    devices = [jax.devices()[i] for i in device_ids]
    mesh = Mesh(np.asarray(devices), ("core",))
    specs = (PartitionSpec("core"),) * (n_params + n_outs)
    sharded = jax.jit(
        shard_map(_body, mesh=mesh, in_specs=specs,
                  out_specs=(PartitionSpec("core"),) * n_outs,
                  check_rep=False),
        donate_argnums=donate, keep_unused=True,
    )
    concat_in = [
        np.concatenate([np.asarray(in_maps[c][nm]) for c in range(n_cores)],
                       axis=0) for nm in in_names
    ]
    concat_zeros = [
        np.zeros((n_cores * z.shape[0], *z.shape[1:]), z.dtype)
        for z in zero_outs
    ]

    trace = os.environ.get("BASS_TRACE") == "1"
    hook = None
    if trace:
        try:
            from antenv.axon_hooks import get_axon_ntff_profile_hook
            hook = get_axon_ntff_profile_hook()
        except ImportError:
            hook = None

    if hook is not None:
        neff_dir = tempfile.mkdtemp()
        with hook(neff_dir, [device_ids[0]]):
            out_arrs = sharded(*concat_in, *concat_zeros)
    else:
        out_arrs = sharded(*concat_in, *concat_zeros)

    results = [
        {nm: np.asarray(out_arrs[i]).reshape(n_cores, *out_avals[i].shape)[c]
         for i, nm in enumerate(out_names)}
        for c in range(n_cores)
    ]

    perf = BassKernelResults(results=results, instructions_and_trace=None,
                             profile_json=None, exec_time_ns=None)
    if hook is not None and glob.glob(os.path.join(neff_dir, "*_body*.ntff")):
        import gauge.profiler
        from concourse._compat import FishPath
        profile = gauge.profiler.Profile(
            profile_path=FishPath(neff_dir), kernel_dev_mode=True,
            profile_on_exit=False, bass_kernel=nc.m,
            offline_processing=True, fname="*_body*",
            metadata={"artifacts_path": neff_dir},
        )
        p = _process_ntff_profile(
            profile, neff_dir, nc, device_ids, [device_ids[0]], False, {},
            trace_events=False)
        perf = p.as_bass_kernel_results(results)
    return perf


def kernel(A, WW, train_init, alpha, fract, lambd, l, A_y_list):
    from concourse.bass_utils import run_bass_kernel_spmd

    if USE_BF16:
        import ml_dtypes
        mm_dt = ml_dtypes.bfloat16
    else:
        mm_dt = np.float32

    A = np.asarray(A, dtype=np.float32)
    WW = np.asarray(WW, dtype=np.float32)
    train_init = np.asarray(train_init, dtype=np.float32)

    P, Q, R = _host_coefs(
        np.asarray(alpha, np.float32), np.asarray(fract, np.float32),
        np.asarray(lambd, np.float32), np.asarray(l, np.float32))

    Wneg = -WW[:, :, 0]                         # [1200, 1200]

    xts, wcs, coefs = {}, {}, {}
    for beta in range(PB):
        bsl = slice(beta * BL, (beta + 1) * BL)
        xa = A[:, bsl, :, 0].transpose(2, 0, 1).reshape(N, CA)      # col=la*BL+b
        xu = train_init[bsl, :, :, 1].transpose(1, 2, 0).reshape(N, CU)  # col=t*BL+b
        XT = np.concatenate([xa, xu], axis=1)                       # [1200, 400]
        # partition-major: [KT, NK*C], col = k*C + c
        xts[beta] = np.ascontiguousarray(
            XT.reshape(NK, KT, C).transpose(1, 0, 2).reshape(KT, NK * C),
            dtype=mm_dt)
    for g in range(PJ):
        gsl = slice(g * JL, (g + 1) * JL)
        if FOUR_CORES:
            # partition-major, jt-major: col = jt*NK*JS + k*JS + s
            wcs[g] = np.ascontiguousarray(
                Wneg[:, gsl].reshape(NK, KT, NJ, JS).transpose(1, 2, 0, 3)
                .reshape(KT, NK * JL), dtype=mm_dt)
        else:
            # partition-major, k-major: col = k*JL + j
            wcs[g] = np.ascontiguousarray(
                Wneg[:, gsl].reshape(NK, KT, JL).transpose(1, 0, 2)
                .reshape(KT, NK * JL), dtype=mm_dt)
        # coef [JS, 108]: col = kind*36 + jt*12 + la
        kinds = [M[:, gsl].reshape(NL, NJ, JS).transpose(2, 1, 0)
                 for M in (P, Q, R)]                                # [100, 3, 12]
        coefs[g] = np.ascontiguousarray(
            np.stack(kinds, axis=1).reshape(JS, 3 * NJ * NL), dtype=np.float32)

    in_maps = []
    for core in range(PB * PJ):
        beta, g = divmod(core, PJ)
        in_maps.append({"xt": xts[beta], "wc": wcs[g], "coef": coefs[g]})

    nc = _get_nc()
    if FOUR_CORES:
        res = _run_on_devices(nc, in_maps, DEVICE_IDS)
    else:
        res = run_bass_kernel_spmd(nc, in_maps, core_ids=list(range(PB * PJ)))
    kernel.last_results = res

    full = np.empty((B, NL, N), dtype=np.float32)
    for core in range(PB * PJ):
        beta, g = divmod(core, PJ)
        o = res.results[core]["out"]            # [300, 192], col = la*BL+b
        full[beta * BL:(beta + 1) * BL, :, g * JL:(g + 1) * JL] = (
            o.reshape(JL, NL, BL).transpose(2, 1, 0))
    return full.reshape(B, NL, N, 1)


# revision 40
# speedup vs baseline: 1.2689x; 1.0048x over previous
# Trainium2 Bass kernel for nn_FCM_series_1 (gnn_message_passing).
#
# Math (derived from the reference):
#   aggregate(X, WW)[l,b,j] = tanh(-sum_i X[l,b,i] * WW[i,j])
#   T_A  = aggregate(A, WW)                     (12 lags x B rows)
#   U[t] = aggregate(train_init[:,:,t,1], WW)   (13 unique rows per batch;
#          A_N_OLD[la] = U[la], A_0_NEW[la] = U[la+1])
#   out[b,la,j] = P[la,j]*T_A[la,b,j] + Q[la,j]*U[la+1,b,j] + R[la,j]*U[la,b,j]
# with host-computable coefficients
#   P[la,j] = 2 * lambd[la, j%200] / belta[la] * 3**fract[la]
#   Q[la,j] = 3 * lambd[la, j%200] * l[la, j%200] / belta[la]
#   R[la,j] = Q[la,j] * Gamma(a+1)/(6*Gamma(a-2))
#   belta[la] = sum_{k=0..3} Gamma(a+1)/(Gamma(k+1)*Gamma(a-k+1))
#
# Sharding over 8 cores: batch split x2 (16 each), output node dim j split x4
# (300 each). Per core one matmul chain: lhsT=W-chunk tiles, rhs=X^T tiles,
# PSUM-accumulated over 10 k-tiles of 120, in float32r (single-pass fp32 PE
# mode, 4x faster than fp32 LOW_HIGH). W is negated on the host so psum
# already holds -X@W; tanh on ScalarE; coefficient combine on VectorE with
# 0-stride broadcast APs; per-core [300,192] result re-assembled on the host.
#
# HBM layouts are host-repacked to partition-major so every DMA descriptor is
# one large contiguous run per partition; input DMAs are split between the two
# HWDGE queues (sync for W, scalar for X) to double aggregate DMA throughput.

import math

import numpy as np

LAG = 13
B = 32
N = 1200
H = 1.0 / 3.0

# FOUR_CORES: run on one NeuronCore per SEngine pair (devices 0,2,4,6).
# Measured WORSE (46.5 us vs 37.3): per-core DMA is capped ~215 GB/s even
# with the paired core idle, so fewer cores just means more bytes per core.
FOUR_CORES = False
PB = 2          # batch shards
PJ = 2 if FOUR_CORES else 4   # j shards
BL = B // PB    # 16 batches per core
JL = N // PJ    # output nodes per core (600 / 300)
NL = LAG - 1    # 12
CA = NL * BL    # 192 cols: T_A block, col = la*BL + b
CU = LAG * BL   # 208 cols: U block,  col = CA + t*BL + b
C = CA + CU     # 400 matmul moving cols
KT = 120        # contraction tile
NK = N // KT    # 10
JS = 100        # j subtile (psum partition dim)
NJ = JL // JS   # j subtiles per core (6 / 3)
NCH = 2         # input DMA chunks per tensor (5 k-tiles each)
USE_BF16 = False  # matmul operand dtype: bf16 halves DMA bytes, ~2.5e-3 rel err
DEVICE_IDS = [0, 2, 4, 6] if FOUR_CORES else list(range(8))

_cached = None


def _gamma(x):
    return math.gamma(x)


def _build_nc():
    import concourse.bacc as bacc
    import concourse.mybir as mybir
    from concourse.tile import TileContext

    f32 = mybir.dt.float32
    f32r = mybir.dt.bfloat16 if USE_BF16 else mybir.dt.float32r
    nc = bacc.Bacc(None, target_bir_lowering=False)

    # partition-major repacked inputs (see kernel() for layouts)
    xt = nc.dram_tensor("xt", [KT, NK * C], f32r, kind="ExternalInput")
    wc = nc.dram_tensor("wc", [KT, NK * JL], f32r, kind="ExternalInput")
    coef = nc.dram_tensor("coef", [JS, 3 * NJ * NL], f32, kind="ExternalInput")
    out = nc.dram_tensor("out", [JL, CA], f32, kind="ExternalOutput")

    with TileContext(nc) as tc:
        with (
            tc.tile_pool(name="sb", bufs=1) as pool,
            tc.tile_pool(name="ps", bufs=1, space="PSUM") as pspool,
        ):
            x_tiles = [None] * NK   # per-k [KT, C] views
            w_tiles = [None] * NK   # 8-core mode: per-k [KT, JL] views
            wjt = [None] * NJ       # 4-core mode: per-jt [KT, NK*JS] views

            def load(eng, kind, k0, nk, gi):
                dram, width, tl = (wc, JL, w_tiles) if kind == "w" \
                    else (xt, C, x_tiles)
                g = pool.tile([KT, nk * width], f32r, tag=f"g{gi}",
                              name=f"g{gi}")
                eng.dma_start(
                    out=g[:], in_=dram[:, k0 * width:(k0 + nk) * width])
                for kk in range(nk):
                    tl[k0 + kk] = g[:, kk * width:(kk + 1) * width]

            if FOUR_CORES:
                # X first on both queues (needed by every jt), then W
                # jt-major so per-jt epilogues pipeline with the W stream.
                # sync: Xk0-4 | Wjt0 | Wjt2 | Wjt4; scalar: Xk5-9 | Wjt1..5
                load(nc.sync, "x", 0, 5, 0)
                load(nc.scalar, "x", 5, 5, 1)
                KJ = NK * JS
                for jt in range(NJ):
                    g = pool.tile([KT, KJ], f32r, tag=f"wj{jt}",
                                  name=f"wj{jt}")
                    eng = nc.sync if jt % 2 == 0 else nc.scalar
                    eng.dma_start(out=g[:], in_=wc[:, jt * KJ:(jt + 1) * KJ])
                    wjt[jt] = g

                def w_slice(jt, k):
                    return wjt[jt][:, k * JS:(k + 1) * JS]
            else:
                # k-major on both queues with a fine-grained tail: the k9
                # weights land last as three per-jt 40 KB pieces, so each
                # j-subtile's tanh+combine chain starts as soon as its own
                # final matmul can run, pipelining the epilogue with the
                # tail of the stream.
                # sync:   Wk0-2 | Wk3-5 | Xk6-7 | Xk8 | Xk9 | Wk9jt0
                # scalar: Xk0-2 | Xk3-5 | Wk6-8 | Wk9jt1 | Wk9jt2
                load(nc.sync, "w", 0, 3, 0)
                load(nc.scalar, "x", 0, 3, 1)
                load(nc.sync, "w", 3, 3, 2)
                load(nc.scalar, "x", 3, 3, 3)
                load(nc.sync, "x", 6, 2, 4)
                load(nc.scalar, "w", 6, 3, 5)
                load(nc.sync, "x", 8, 1, 6)
                load(nc.scalar, "x", 9, 1, 7)
                wk9 = []
                for jt in range(NJ):
                    g = pool.tile([KT, JS], f32r, tag=f"wk9{jt}",
                                  name=f"wk9{jt}")
                    eng = nc.sync if jt == 0 else nc.scalar
                    eng.dma_start(
                        out=g[:],
                        in_=wc[:, 9 * JL + jt * JS:9 * JL + (jt + 1) * JS])
                    wk9.append(g)

                def w_slice(jt, k):
                    if k == NK - 1:
                        return wk9[jt][:]
                    return w_tiles[k][:, jt * JS:(jt + 1) * JS]
            coef_all = pool.tile([JS, 3 * NJ * NL], f32, tag="coef")
            nc.gpsimd.dma_start(out=coef_all[:], in_=coef[:, :])

            # Warm up the PE clock gate (HAM) with throwaway bf16 matmuls
            # while inputs stream: otherwise the ~22 matmuls left after the
            # last DMA chunk run at the cold 333 ns cadence instead of 180.
            bf16 = mybir.dt.bfloat16
            scratch = pool.tile([KT, C], bf16, tag="scr")
            nc.vector.memset(scratch[:], 0)
            psw = pspool.tile([JS, C], f32, tag="psw", name="psw")
            for i in range(36):
                nc.tensor.matmul(psw[:], scratch[:, 0:JS], scratch[:],
                                 start=True, stop=True)

            ps = [pspool.tile([JS, C], f32, tag=f"ps{jt}", name=f"ps{jt}")
                  for jt in range(NJ)]
            if FOUR_CORES:
                mm_order = [(jt, k) for jt in range(NJ) for k in range(NK)]
            else:
                mm_order = [(jt, k) for k in range(NK) for jt in range(NJ)]
            # Dummy-matmul gap fillers after these k-groups keep the PE busy
            # through mid-stream DMA stalls so HAM stays at 2.4 GHz (a >3.4us
            # idle window re-throttles it to 1.2 GHz).
            fill_after = {2: 8, 5: 8, 8: 6}
            for jt, k in mm_order:
                nc.tensor.matmul(
                    ps[jt][:], w_slice(jt, k), x_tiles[k],
                    start=(k == 0), stop=(k == NK - 1),
                )
                if jt == NJ - 1 and k in fill_after:
                    for _ in range(fill_after[k]):
                        nc.tensor.matmul(psw[:], scratch[:, 0:JS],
                                         scratch[:], start=True, stop=True)

            # Replicate the [JS, 12] coefficient vectors to [JS, 192] during
            # the DMA phase (DVE idle) so the combine ops run on flat APs.
            crep = pool.tile([JS, 3 * NJ * CA], f32, tag="crep")
            for i in range(3):
                for jt in range(NJ):
                    src = coef_all[:, i * NJ * NL + jt * NL:
                                   i * NJ * NL + (jt + 1) * NL]
                    dst = crep[:, (i * NJ + jt) * CA:(i * NJ + jt + 1) * CA]
                    # DVE is idle during the DMA phase; GpSimd would contend
                    # for the shared DVE/GpSimd SBUF port later.
                    nc.vector.tensor_copy(
                        dst.rearrange("p (l b) -> p l b", b=BL),
                        src.broadcast_to([JS, NL, BL]))

            # Per-jt epilogue, pipelined: tanh on ACT, flat combine on DVE
            # (jt0, jt2) / GpSimd (jt1), per-jt output DMA.
            t_all = pool.tile([JS, NJ * C], f32, tag="t")
            res = pool.tile([JS, NJ * CA], f32, tag="res")
            tmp = pool.tile([JS, NJ * CA], f32, tag="tmp")
            tmp2 = pool.tile([JS, NJ * CA], f32, tag="tmp2")
            out3 = out.rearrange("(j p) c -> p j c", p=JS)
            for jt in range(NJ):
                # W was negated on the host, so psum = -(X @ W) already.
                nc.scalar.activation(
                    out=t_all[:, jt * C:(jt + 1) * C], in_=ps[jt][:],
                    func=mybir.ActivationFunctionType.Tanh,
                )
                t0 = jt * C
                tA = t_all[:, t0:t0 + CA]
                tU1 = t_all[:, t0 + CA + BL:t0 + CA + CU]
                tU0 = t_all[:, t0 + CA:t0 + CA + CA]
                rs = res[:, jt * CA:(jt + 1) * CA]
                ts = tmp[:, jt * CA:(jt + 1) * CA]
                ts2 = tmp2[:, jt * CA:(jt + 1) * CA]
                cof = [crep[:, (i * NJ + jt) * CA:(i * NJ + jt + 1) * CA]
                       for i in range(3)]
                # All combines on DVE: GpSimd shares the DVE SBUF port pair,
                # so "parallel" gp combines just halve both engines' rates.
                ve = nc.vector
                # three independent muls (pipeline on the engine), then adds
                ve.tensor_mul(rs, cof[0], tA)
                ve.tensor_mul(ts, cof[1], tU1)
                ve.tensor_mul(ts2, cof[2], tU0)
                ve.tensor_add(rs, rs, ts)
                ve.tensor_add(rs, rs, ts2)
                oeng = nc.sync if jt != 1 else nc.scalar
                oeng.dma_start(out=out3[:, jt, :], in_=rs)

    return nc


def _get_nc():
    global _cached
    if _cached is None:
        _cached = _build_nc()
        _cached.finalize()   # Bacc: runs reg alloc + codegen passes
    return _cached


def _host_coefs(alpha, fract, lambd, l):
    # All [12,...] fp32; compute in float64, cast at the end.
    a = alpha[:, 0].astype(np.float64)          # [12]
    f = fract[:, 0].astype(np.float64)          # [12]
    lam = lambd[:, 0, :, 0].astype(np.float64)  # [12, 200]
    ll = l[:, 0, :, 0].astype(np.float64)       # [12, 200]

    belta = np.zeros(NL)
    for la in range(NL):
        g_a1 = _gamma(a[la] + 1.0)
        belta[la] = sum(
            g_a1 / (_gamma(kk + 1.0) * _gamma(a[la] - kk + 1.0)) for kk in range(4)
        )
    cN = np.array([_gamma(a[la] + 1.0) / (6.0 * _gamma(a[la] - 2.0))
                   for la in range(NL)])

    # tile lambda/l from 200 -> 1200 (index n % 200)
    lam_t = np.tile(lam, (1, 6))                # [12, 1200]
    ll_t = np.tile(ll, (1, 6))                  # [12, 1200]

    inv_hf = (1.0 / H) ** f                     # 3**fract
    P = 2.0 * lam_t / belta[:, None] * inv_hf[:, None]
    Q = lam_t * ll_t / belta[:, None] / H
    R = Q * cN[:, None]
    return P.astype(np.float32), Q.astype(np.float32), R.astype(np.float32)


def _run_on_devices(nc, in_maps, device_ids):
    """run_bass_via_pjrt with an explicit device list (one core per SEngine
    pair) plus optional NTFF profiling. Returns a BassKernelResults."""
    import glob
    import os
    import tempfile

    import jax
    from jax.sharding import Mesh, PartitionSpec
    from jax.experimental.shard_map import shard_map

    import concourse.mybir as mybir
    from concourse.bass2jax import _bass_exec_p, install_neuronx_cc_hook
    from concourse.bass_utils import BassKernelResults, _process_ntff_profile

    install_neuronx_cc_hook()
    n_cores = len(device_ids)
    part_name = (nc.partition_id_tensor.name
                 if nc.partition_id_tensor else None)

    in_names, out_names, out_avals, zero_outs = [], [], [], []
    for alloc in nc.m.functions[0].allocations:
        if not isinstance(alloc, mybir.MemoryLocationSet):
            continue
        name = alloc.memorylocations[0].name
        if alloc.kind == "ExternalInput":
            if name != part_name:
                in_names.append(name)
        elif alloc.kind == "ExternalOutput":
            shape = tuple(alloc.tensor_shape)
            dtype = mybir.dt.np(alloc.dtype)
            out_names.append(name)
            out_avals.append(jax.core.ShapedArray(shape, dtype))
            zero_outs.append(np.zeros(shape, dtype))
    n_params = len(in_names)
    n_outs = len(out_avals)
    all_names = in_names + out_names
    if part_name is not None:
        all_names = all_names + [part_name]
    donate = tuple(range(n_params, n_params + n_outs))

    def _body(*args):
        operands = list(args)
        if part_name is not None:
            from concourse.bass2jax import partition_id_tensor
            operands.append(partition_id_tensor())
        outs = _bass_exec_p.bind(
            *operands,
            out_avals=tuple(out_avals),
            in_names=tuple(all_names),
            out_names=tuple(out_names),
            lowering_input_output_aliases=(),
            sim_require_finite=True,
            sim_require_nnan=True,
            nc=nc,
        )
        return tuple(outs)

    devices = [jax.devices()[i] for i in device_ids]
    mesh = Mesh(np.asarray(devices), ("core",))
    specs = (PartitionSpec("core"),) * (n_params + n_outs)
    sharded = jax.jit(
        shard_map(_body, mesh=mesh, in_specs=specs,
                  out_specs=(PartitionSpec("core"),) * n_outs,
                  check_rep=False),
        donate_argnums=donate, keep_unused=True,
    )
    concat_in = [
        np.concatenate([np.asarray(in_maps[c][nm]) for c in range(n_cores)],
                       axis=0) for nm in in_names
    ]
    concat_zeros = [
        np.zeros((n_cores * z.shape[0], *z.shape[1:]), z.dtype)
        for z in zero_outs
    ]

    trace = os.environ.get("BASS_TRACE") == "1"
    hook = None
    if trace:
        try:
            from antenv.axon_hooks import get_axon_ntff_profile_hook
            hook = get_axon_ntff_profile_hook()
        except ImportError:
            hook = None

    if hook is not None:
        neff_dir = tempfile.mkdtemp()
        with hook(neff_dir, [device_ids[0]]):
            out_arrs = sharded(*concat_in, *concat_zeros)
    else:
        out_arrs = sharded(*concat_in, *concat_zeros)

    results = [
        {nm: np.asarray(out_arrs[i]).reshape(n_cores, *out_avals[i].shape)[c]
         for i, nm in enumerate(out_names)}
        for c in range(n_cores)
    ]

    perf = BassKernelResults(results=results, instructions_and_trace=None,
                             profile_json=None, exec_time_ns=None)
    if hook is not None and glob.glob(os.path.join(neff_dir, "*_body*.ntff")):
        import gauge.profiler
        from concourse._compat import FishPath
        profile = gauge.profiler.Profile(
            profile_path=FishPath(neff_dir), kernel_dev_mode=True,
            profile_on_exit=False, bass_kernel=nc.m,
            offline_processing=True, fname="*_body*",
            metadata={"artifacts_path": neff_dir},
        )
        p = _process_ntff_profile(
            profile, neff_dir, nc, device_ids, [device_ids[0]], False, {},
            trace_events=False)
        perf = p.as_bass_kernel_results(results)
    return perf


def kernel(A, WW, train_init, alpha, fract, lambd, l, A_y_list):
    from concourse.bass_utils import run_bass_kernel_spmd

    if USE_BF16:
        import ml_dtypes
        mm_dt = ml_dtypes.bfloat16
    else:
        mm_dt = np.float32

    A = np.asarray(A, dtype=np.float32)
    WW = np.asarray(WW, dtype=np.float32)
    train_init = np.asarray(train_init, dtype=np.float32)

    P, Q, R = _host_coefs(
        np.asarray(alpha, np.float32), np.asarray(fract, np.float32),
        np.asarray(lambd, np.float32), np.asarray(l, np.float32))

    Wneg = -WW[:, :, 0]                         # [1200, 1200]

    xts, wcs, coefs = {}, {}, {}
    for beta in range(PB):
        bsl = slice(beta * BL, (beta + 1) * BL)
        xa = A[:, bsl, :, 0].transpose(2, 0, 1).reshape(N, CA)      # col=la*BL+b
        xu = train_init[bsl, :, :, 1].transpose(1, 2, 0).reshape(N, CU)  # col=t*BL+b
        XT = np.concatenate([xa, xu], axis=1)                       # [1200, 400]
        # partition-major: [KT, NK*C], col = k*C + c
        xts[beta] = np.ascontiguousarray(
            XT.reshape(NK, KT, C).transpose(1, 0, 2).reshape(KT, NK * C),
            dtype=mm_dt)
    for g in range(PJ):
        gsl = slice(g * JL, (g + 1) * JL)
        if FOUR_CORES:
            # partition-major, jt-major: col = jt*NK*JS + k*JS + s
            wcs[g] = np.ascontiguousarray(
                Wneg[:, gsl].reshape(NK, KT, NJ, JS).transpose(1, 2, 0, 3)
                .reshape(KT, NK * JL), dtype=mm_dt)
        else:
            # partition-major, k-major: col = k*JL + j
            wcs[g] = np.ascontiguousarray(
                Wneg[:, gsl].reshape(NK, KT, JL).transpose(1, 0, 2)
                .reshape(KT, NK * JL), dtype=mm_dt)
        # coef [JS, 108]: col = kind*36 + jt*12 + la
        kinds = [M[:, gsl].reshape(NL, NJ, JS).transpose(2, 1, 0)
                 for M in (P, Q, R)]                                # [100, 3, 12]
        coefs[g] = np.ascontiguousarray(
            np.stack(kinds, axis=1).reshape(JS, 3 * NJ * NL), dtype=np.float32)

    in_maps = []
    for core in range(PB * PJ):
        beta, g = divmod(core, PJ)
        in_maps.append({"xt": xts[beta], "wc": wcs[g], "coef": coefs[g]})

    nc = _get_nc()
    if FOUR_CORES:
        res = _run_on_devices(nc, in_maps, DEVICE_IDS)
    else:
        res = run_bass_kernel_spmd(nc, in_maps, core_ids=list(range(PB * PJ)))
    kernel.last_results = res

    full = np.empty((B, NL, N), dtype=np.float32)
    for core in range(PB * PJ):
        beta, g = divmod(core, PJ)
        o = res.results[core]["out"]            # [300, 192], col = la*BL+b
        full[beta * BL:(beta + 1) * BL, :, g * JL:(g + 1) * JL] = (
            o.reshape(JL, NL, BL).transpose(2, 1, 0))
    return full.reshape(B, NL, N, 1)


# revision 43
# speedup vs baseline: 1.2903x; 1.0168x over previous
# Trainium2 Bass kernel for nn_FCM_series_1 (gnn_message_passing).
#
# Math (derived from the reference):
#   aggregate(X, WW)[l,b,j] = tanh(-sum_i X[l,b,i] * WW[i,j])
#   T_A  = aggregate(A, WW)                     (12 lags x B rows)
#   U[t] = aggregate(train_init[:,:,t,1], WW)   (13 unique rows per batch;
#          A_N_OLD[la] = U[la], A_0_NEW[la] = U[la+1])
#   out[b,la,j] = P[la,j]*T_A[la,b,j] + Q[la,j]*U[la+1,b,j] + R[la,j]*U[la,b,j]
# with host-computable coefficients
#   P[la,j] = 2 * lambd[la, j%200] / belta[la] * 3**fract[la]
#   Q[la,j] = 3 * lambd[la, j%200] * l[la, j%200] / belta[la]
#   R[la,j] = Q[la,j] * Gamma(a+1)/(6*Gamma(a-2))
#   belta[la] = sum_{k=0..3} Gamma(a+1)/(Gamma(k+1)*Gamma(a-k+1))
#
# Sharding over 8 cores: batch split x2 (16 each), output node dim j split x4
# (300 each). Per core one matmul chain: lhsT=W-chunk tiles, rhs=X^T tiles,
# PSUM-accumulated over 10 k-tiles of 120, in float32r (single-pass fp32 PE
# mode, 4x faster than fp32 LOW_HIGH). W is negated on the host so psum
# already holds -X@W; tanh on ScalarE; coefficient combine on VectorE with
# 0-stride broadcast APs; per-core [300,192] result re-assembled on the host.
#
# HBM layouts are host-repacked to partition-major so every DMA descriptor is
# one large contiguous run per partition; input DMAs are split between the two
# HWDGE queues (sync for W, scalar for X) to double aggregate DMA throughput.

import math

import numpy as np

LAG = 13
B = 32
N = 1200
H = 1.0 / 3.0

# FOUR_CORES: run on one NeuronCore per SEngine pair (devices 0,2,4,6).
# Measured WORSE (46.5 us vs 37.3): per-core DMA is capped ~215 GB/s even
# with the paired core idle, so fewer cores just means more bytes per core.
FOUR_CORES = False
PB = 2          # batch shards
PJ = 2 if FOUR_CORES else 4   # j shards
BL = B // PB    # 16 batches per core
JL = N // PJ    # output nodes per core (600 / 300)
NL = LAG - 1    # 12
CA = NL * BL    # 192 cols: T_A block, col = la*BL + b
CU = LAG * BL   # 208 cols: U block,  col = CA + t*BL + b
C = CA + CU     # 400 matmul moving cols
KT = 120        # contraction tile
NK = N // KT    # 10
JS = 100        # j subtile (psum partition dim)
NJ = JL // JS   # j subtiles per core (6 / 3)
NCH = 2         # input DMA chunks per tensor (5 k-tiles each)
USE_BF16 = False  # matmul operand dtype: bf16 halves DMA bytes, ~2.5e-3 rel err
DEVICE_IDS = [0, 2, 4, 6] if FOUR_CORES else list(range(8))

_cached = None


def _gamma(x):
    return math.gamma(x)


def _build_nc():
    import concourse.bacc as bacc
    import concourse.mybir as mybir
    from concourse.tile import TileContext

    f32 = mybir.dt.float32
    f32r = mybir.dt.bfloat16 if USE_BF16 else mybir.dt.float32r
    nc = bacc.Bacc(None, target_bir_lowering=False)

    # partition-major repacked inputs (see kernel() for layouts)
    xt = nc.dram_tensor("xt", [KT, NK * C], f32r, kind="ExternalInput")
    wc = nc.dram_tensor("wc", [KT, NK * JL], f32r, kind="ExternalInput")
    coef = nc.dram_tensor("coef", [JS, 3 * NJ * NL], f32, kind="ExternalInput")
    out = nc.dram_tensor("out", [JL, CA], f32, kind="ExternalOutput")

    with TileContext(nc) as tc:
        with (
            tc.tile_pool(name="sb", bufs=1) as pool,
            tc.tile_pool(name="ps", bufs=1, space="PSUM") as pspool,
        ):
            x_tiles = [None] * NK   # per-k [KT, C] views
            w_tiles = [None] * NK   # 8-core mode: per-k [KT, JL] views
            wjt = [None] * NJ       # 4-core mode: per-jt [KT, NK*JS] views

            def load(eng, kind, k0, nk, gi):
                dram, width, tl = (wc, JL, w_tiles) if kind == "w" \
                    else (xt, C, x_tiles)
                g = pool.tile([KT, nk * width], f32r, tag=f"g{gi}",
                              name=f"g{gi}")
                eng.dma_start(
                    out=g[:], in_=dram[:, k0 * width:(k0 + nk) * width])
                for kk in range(nk):
                    tl[k0 + kk] = g[:, kk * width:(kk + 1) * width]

            if FOUR_CORES:
                # X first on both queues (needed by every jt), then W
                # jt-major so per-jt epilogues pipeline with the W stream.
                # sync: Xk0-4 | Wjt0 | Wjt2 | Wjt4; scalar: Xk5-9 | Wjt1..5
                load(nc.sync, "x", 0, 5, 0)
                load(nc.scalar, "x", 5, 5, 1)
                KJ = NK * JS
                for jt in range(NJ):
                    g = pool.tile([KT, KJ], f32r, tag=f"wj{jt}",
                                  name=f"wj{jt}")
                    eng = nc.sync if jt % 2 == 0 else nc.scalar
                    eng.dma_start(out=g[:], in_=wc[:, jt * KJ:(jt + 1) * KJ])
                    wjt[jt] = g

                def w_slice(jt, k):
                    return wjt[jt][:, k * JS:(k + 1) * JS]
            else:
                # k-major on both queues with a per-jt staggered tail: W's
                # k8+k9 blocks are host-repacked per jt as contiguous 96 KB
                # pieces landing last, so jt0/jt1/jt2 finish ~0.5us apart
                # and each tanh+combine chain starts as soon as possible.
                # sync:   Wk0-2 | Wk3-5 | Xk6-7 | Xk8 | Xk9 | Wtail-jt0
                # scalar: Xk0-2 | Xk3-5 | Wk6-7 | Wtail-jt1 | Wtail-jt2
                load(nc.sync, "w", 0, 3, 0)
                load(nc.scalar, "x", 0, 3, 1)
                load(nc.sync, "w", 3, 3, 2)
                load(nc.scalar, "x", 3, 3, 3)
                load(nc.sync, "x", 6, 2, 4)
                load(nc.scalar, "w", 6, 2, 5)
                load(nc.sync, "x", 8, 1, 6)
                load(nc.scalar, "x", 9, 1, 7)
                wtail = []
                for jt in range(NJ):
                    g = pool.tile([KT, 2 * JS], f32r, tag=f"wt{jt}",
                                  name=f"wt{jt}")
                    eng = nc.sync if jt == 0 else nc.scalar
                    c0 = 8 * JL + jt * 2 * JS
                    eng.dma_start(out=g[:], in_=wc[:, c0:c0 + 2 * JS])
                    wtail.append(g)

                def w_slice(jt, k):
                    if k >= 8:
                        return wtail[jt][:, (k - 8) * JS:(k - 7) * JS]
                    return w_tiles[k][:, jt * JS:(jt + 1) * JS]
            coef_all = pool.tile([JS, 3 * NJ * NL], f32, tag="coef")
            nc.gpsimd.dma_start(out=coef_all[:], in_=coef[:, :])

            # Warm up the PE clock gate (HAM) with throwaway bf16 matmuls
            # while inputs stream: otherwise the ~22 matmuls left after the
            # last DMA chunk run at the cold 333 ns cadence instead of 180.
            bf16 = mybir.dt.bfloat16
            scratch = pool.tile([KT, C], bf16, tag="scr")
            nc.vector.memset(scratch[:], 0)
            psw = pspool.tile([JS, C], f32, tag="psw", name="psw")
            for i in range(36):
                nc.tensor.matmul(psw[:], scratch[:, 0:JS], scratch[:],
                                 start=True, stop=True)

            ps = [pspool.tile([JS, C], f32, tag=f"ps{jt}", name=f"ps{jt}")
                  for jt in range(NJ)]
            if FOUR_CORES:
                mm_order = [(jt, k) for jt in range(NJ) for k in range(NK)]
            else:
                # k0-7 k-outer, then per-jt (k8, k9) so jt completions
                # stagger with the per-jt W tail pieces.
                mm_order = [(jt, k) for k in range(8) for jt in range(NJ)]
                mm_order += [(jt, k) for jt in range(NJ) for k in (8, 9)]
            # Dummy-matmul gap fillers after these k-groups keep the PE busy
            # through mid-stream DMA stalls so HAM stays at 2.4 GHz (a >3.4us
            # idle window re-throttles it to 1.2 GHz).
            fill_after = {2: 8, 5: 8}
            for jt, k in mm_order:
                nc.tensor.matmul(
                    ps[jt][:], w_slice(jt, k), x_tiles[k],
                    start=(k == 0), stop=(k == NK - 1),
                )
                if jt == NJ - 1 and k in fill_after:
                    for _ in range(fill_after[k]):
                        nc.tensor.matmul(psw[:], scratch[:, 0:JS],
                                         scratch[:], start=True, stop=True)

            # Replicate the [JS, 12] coefficient vectors to [JS, 192] during
            # the DMA phase (DVE idle) so the combine ops run on flat APs.
            crep = pool.tile([JS, 3 * NJ * CA], f32, tag="crep")
            for i in range(3):
                for jt in range(NJ):
                    src = coef_all[:, i * NJ * NL + jt * NL:
                                   i * NJ * NL + (jt + 1) * NL]
                    dst = crep[:, (i * NJ + jt) * CA:(i * NJ + jt + 1) * CA]
                    # DVE is idle during the DMA phase; GpSimd would contend
                    # for the shared DVE/GpSimd SBUF port later.
                    nc.vector.tensor_copy(
                        dst.rearrange("p (l b) -> p l b", b=BL),
                        src.broadcast_to([JS, NL, BL]))

            # Per-jt epilogue, pipelined: tanh on ACT, flat combine on DVE
            # (jt0, jt2) / GpSimd (jt1), per-jt output DMA.
            t_all = pool.tile([JS, NJ * C], f32, tag="t")
            res = pool.tile([JS, NJ * CA], f32, tag="res")
            tmp = pool.tile([JS, NJ * CA], f32, tag="tmp")
            tmp2 = pool.tile([JS, NJ * CA], f32, tag="tmp2")
            out3 = out.rearrange("(j p) c -> p j c", p=JS)
            for jt in range(NJ):
                # W was negated on the host, so psum = -(X @ W) already.
                nc.scalar.activation(
                    out=t_all[:, jt * C:(jt + 1) * C], in_=ps[jt][:],
                    func=mybir.ActivationFunctionType.Tanh,
                )
                t0 = jt * C
                tA = t_all[:, t0:t0 + CA]
                tU1 = t_all[:, t0 + CA + BL:t0 + CA + CU]
                tU0 = t_all[:, t0 + CA:t0 + CA + CA]
                rs = res[:, jt * CA:(jt + 1) * CA]
                ts = tmp[:, jt * CA:(jt + 1) * CA]
                ts2 = tmp2[:, jt * CA:(jt + 1) * CA]
                cof = [crep[:, (i * NJ + jt) * CA:(i * NJ + jt + 1) * CA]
                       for i in range(3)]
                # All combines on DVE: GpSimd shares the DVE SBUF port pair,
                # so "parallel" gp combines just halve both engines' rates.
                ve = nc.vector
                # three independent muls (pipeline on the engine), then adds
                ve.tensor_mul(rs, cof[0], tA)
                ve.tensor_mul(ts, cof[1], tU1)
                ve.tensor_mul(ts2, cof[2], tU0)
                ve.tensor_add(rs, rs, ts)
                ve.tensor_add(rs, rs, ts2)
                oeng = nc.sync if jt != 1 else nc.scalar
                oeng.dma_start(out=out3[:, jt, :], in_=rs)

    return nc


def _get_nc():
    global _cached
    if _cached is None:
        _cached = _build_nc()
        _cached.finalize()   # Bacc: runs reg alloc + codegen passes
    return _cached


def _host_coefs(alpha, fract, lambd, l):
    # All [12,...] fp32; compute in float64, cast at the end.
    a = alpha[:, 0].astype(np.float64)          # [12]
    f = fract[:, 0].astype(np.float64)          # [12]
    lam = lambd[:, 0, :, 0].astype(np.float64)  # [12, 200]
    ll = l[:, 0, :, 0].astype(np.float64)       # [12, 200]

    belta = np.zeros(NL)
    for la in range(NL):
        g_a1 = _gamma(a[la] + 1.0)
        belta[la] = sum(
            g_a1 / (_gamma(kk + 1.0) * _gamma(a[la] - kk + 1.0)) for kk in range(4)
        )
    cN = np.array([_gamma(a[la] + 1.0) / (6.0 * _gamma(a[la] - 2.0))
                   for la in range(NL)])

    # tile lambda/l from 200 -> 1200 (index n % 200)
    lam_t = np.tile(lam, (1, 6))                # [12, 1200]
    ll_t = np.tile(ll, (1, 6))                  # [12, 1200]

    inv_hf = (1.0 / H) ** f                     # 3**fract
    P = 2.0 * lam_t / belta[:, None] * inv_hf[:, None]
    Q = lam_t * ll_t / belta[:, None] / H
    R = Q * cN[:, None]
    return P.astype(np.float32), Q.astype(np.float32), R.astype(np.float32)


def _run_on_devices(nc, in_maps, device_ids):
    """run_bass_via_pjrt with an explicit device list (one core per SEngine
    pair) plus optional NTFF profiling. Returns a BassKernelResults."""
    import glob
    import os
    import tempfile

    import jax
    from jax.sharding import Mesh, PartitionSpec
    from jax.experimental.shard_map import shard_map

    import concourse.mybir as mybir
    from concourse.bass2jax import _bass_exec_p, install_neuronx_cc_hook
    from concourse.bass_utils import BassKernelResults, _process_ntff_profile

    install_neuronx_cc_hook()
    n_cores = len(device_ids)
    part_name = (nc.partition_id_tensor.name
                 if nc.partition_id_tensor else None)

    in_names, out_names, out_avals, zero_outs = [], [], [], []
    for alloc in nc.m.functions[0].allocations:
        if not isinstance(alloc, mybir.MemoryLocationSet):
            continue
        name = alloc.memorylocations[0].name
        if alloc.kind == "ExternalInput":
            if name != part_name:
                in_names.append(name)
        elif alloc.kind == "ExternalOutput":
            shape = tuple(alloc.tensor_shape)
            dtype = mybir.dt.np(alloc.dtype)
            out_names.append(name)
            out_avals.append(jax.core.ShapedArray(shape, dtype))
            zero_outs.append(np.zeros(shape, dtype))
    n_params = len(in_names)
    n_outs = len(out_avals)
    all_names = in_names + out_names
    if part_name is not None:
        all_names = all_names + [part_name]
    donate = tuple(range(n_params, n_params + n_outs))

    def _body(*args):
        operands = list(args)
        if part_name is not None:
            from concourse.bass2jax import partition_id_tensor
            operands.append(partition_id_tensor())
        outs = _bass_exec_p.bind(
            *operands,
            out_avals=tuple(out_avals),
            in_names=tuple(all_names),
            out_names=tuple(out_names),
            lowering_input_output_aliases=(),
            sim_require_finite=True,
            sim_require_nnan=True,
            nc=nc,
        )
        return tuple(outs)

    devices = [jax.devices()[i] for i in device_ids]
    mesh = Mesh(np.asarray(devices), ("core",))
    specs = (PartitionSpec("core"),) * (n_params + n_outs)
    sharded = jax.jit(
        shard_map(_body, mesh=mesh, in_specs=specs,
                  out_specs=(PartitionSpec("core"),) * n_outs,
                  check_rep=False),
        donate_argnums=donate, keep_unused=True,
    )
    concat_in = [
        np.concatenate([np.asarray(in_maps[c][nm]) for c in range(n_cores)],
                       axis=0) for nm in in_names
    ]
    concat_zeros = [
        np.zeros((n_cores * z.shape[0], *z.shape[1:]), z.dtype)
        for z in zero_outs
    ]

    trace = os.environ.get("BASS_TRACE") == "1"
    hook = None
    if trace:
        try:
            from antenv.axon_hooks import get_axon_ntff_profile_hook
            hook = get_axon_ntff_profile_hook()
        except ImportError:
            hook = None

    if hook is not None:
        neff_dir = tempfile.mkdtemp()
        with hook(neff_dir, [device_ids[0]]):
            out_arrs = sharded(*concat_in, *concat_zeros)
    else:
        out_arrs = sharded(*concat_in, *concat_zeros)

    results = [
        {nm: np.asarray(out_arrs[i]).reshape(n_cores, *out_avals[i].shape)[c]
         for i, nm in enumerate(out_names)}
        for c in range(n_cores)
    ]

    perf = BassKernelResults(results=results, instructions_and_trace=None,
                             profile_json=None, exec_time_ns=None)
    if hook is not None and glob.glob(os.path.join(neff_dir, "*_body*.ntff")):
        import gauge.profiler
        from concourse._compat import FishPath
        profile = gauge.profiler.Profile(
            profile_path=FishPath(neff_dir), kernel_dev_mode=True,
            profile_on_exit=False, bass_kernel=nc.m,
            offline_processing=True, fname="*_body*",
            metadata={"artifacts_path": neff_dir},
        )
        p = _process_ntff_profile(
            profile, neff_dir, nc, device_ids, [device_ids[0]], False, {},
            trace_events=False)
        perf = p.as_bass_kernel_results(results)
    return perf


def kernel(A, WW, train_init, alpha, fract, lambd, l, A_y_list):
    from concourse.bass_utils import run_bass_kernel_spmd

    if USE_BF16:
        import ml_dtypes
        mm_dt = ml_dtypes.bfloat16
    else:
        mm_dt = np.float32

    A = np.asarray(A, dtype=np.float32)
    WW = np.asarray(WW, dtype=np.float32)
    train_init = np.asarray(train_init, dtype=np.float32)

    P, Q, R = _host_coefs(
        np.asarray(alpha, np.float32), np.asarray(fract, np.float32),
        np.asarray(lambd, np.float32), np.asarray(l, np.float32))

    Wneg = -WW[:, :, 0]                         # [1200, 1200]

    xts, wcs, coefs = {}, {}, {}
    for beta in range(PB):
        bsl = slice(beta * BL, (beta + 1) * BL)
        xa = A[:, bsl, :, 0].transpose(2, 0, 1).reshape(N, CA)      # col=la*BL+b
        xu = train_init[bsl, :, :, 1].transpose(1, 2, 0).reshape(N, CU)  # col=t*BL+b
        XT = np.concatenate([xa, xu], axis=1)                       # [1200, 400]
        # partition-major: [KT, NK*C], col = k*C + c
        xts[beta] = np.ascontiguousarray(
            XT.reshape(NK, KT, C).transpose(1, 0, 2).reshape(KT, NK * C),
            dtype=mm_dt)
    for g in range(PJ):
        gsl = slice(g * JL, (g + 1) * JL)
        if FOUR_CORES:
            # partition-major, jt-major: col = jt*NK*JS + k*JS + s
            wcs[g] = np.ascontiguousarray(
                Wneg[:, gsl].reshape(NK, KT, NJ, JS).transpose(1, 2, 0, 3)
                .reshape(KT, NK * JL), dtype=mm_dt)
        else:
            # partition-major, k-major for k0-7, then per-jt (k8,k9) tails:
            # cols [k*JL + j for k<8] ++ [8*JL + jt*2*JS + (k-8)*JS + s]
            W3 = Wneg[:, gsl].reshape(NK, KT, JL)
            head = W3[:8].transpose(1, 0, 2).reshape(KT, 8 * JL)
            tails = [W3[k][:, jt * JS:(jt + 1) * JS]
                     for jt in range(NJ) for k in (8, 9)]
            wcs[g] = np.ascontiguousarray(
                np.concatenate([head] + tails, axis=1), dtype=mm_dt)
        # coef [JS, 108]: col = kind*36 + jt*12 + la
        kinds = [M[:, gsl].reshape(NL, NJ, JS).transpose(2, 1, 0)
                 for M in (P, Q, R)]                                # [100, 3, 12]
        coefs[g] = np.ascontiguousarray(
            np.stack(kinds, axis=1).reshape(JS, 3 * NJ * NL), dtype=np.float32)

    in_maps = []
    for core in range(PB * PJ):
        beta, g = divmod(core, PJ)
        in_maps.append({"xt": xts[beta], "wc": wcs[g], "coef": coefs[g]})

    nc = _get_nc()
    if FOUR_CORES:
        res = _run_on_devices(nc, in_maps, DEVICE_IDS)
    else:
        res = run_bass_kernel_spmd(nc, in_maps, core_ids=list(range(PB * PJ)))
    kernel.last_results = res

    full = np.empty((B, NL, N), dtype=np.float32)
    for core in range(PB * PJ):
        beta, g = divmod(core, PJ)
        o = res.results[core]["out"]            # [300, 192], col = la*BL+b
        full[beta * BL:(beta + 1) * BL, :, g * JL:(g + 1) * JL] = (
            o.reshape(JL, NL, BL).transpose(2, 1, 0))
    return full.reshape(B, NL, N, 1)


# revision 44
# speedup vs baseline: 1.3047x; 1.0112x over previous
# Trainium2 Bass kernel for nn_FCM_series_1 (gnn_message_passing).
#
# Math (derived from the reference):
#   aggregate(X, WW)[l,b,j] = tanh(-sum_i X[l,b,i] * WW[i,j])
#   T_A  = aggregate(A, WW)                     (12 lags x B rows)
#   U[t] = aggregate(train_init[:,:,t,1], WW)   (13 unique rows per batch;
#          A_N_OLD[la] = U[la], A_0_NEW[la] = U[la+1])
#   out[b,la,j] = P[la,j]*T_A[la,b,j] + Q[la,j]*U[la+1,b,j] + R[la,j]*U[la,b,j]
# with host-computable coefficients
#   P[la,j] = 2 * lambd[la, j%200] / belta[la] * 3**fract[la]
#   Q[la,j] = 3 * lambd[la, j%200] * l[la, j%200] / belta[la]
#   R[la,j] = Q[la,j] * Gamma(a+1)/(6*Gamma(a-2))
#   belta[la] = sum_{k=0..3} Gamma(a+1)/(Gamma(k+1)*Gamma(a-k+1))
#
# Sharding over 8 cores: batch split x2 (16 each), output node dim j split x4
# (300 each). Per core one matmul chain: lhsT=W-chunk tiles, rhs=X^T tiles,
# PSUM-accumulated over 10 k-tiles of 120, in float32r (single-pass fp32 PE
# mode, 4x faster than fp32 LOW_HIGH). W is negated on the host so psum
# already holds -X@W; tanh on ScalarE; coefficient combine on VectorE with
# 0-stride broadcast APs; per-core [300,192] result re-assembled on the host.
#
# HBM layouts are host-repacked to partition-major so every DMA descriptor is
# one large contiguous run per partition; input DMAs are split between the two
# HWDGE queues (sync for W, scalar for X) to double aggregate DMA throughput.

import math

import numpy as np

LAG = 13
B = 32
N = 1200
H = 1.0 / 3.0

# FOUR_CORES: run on one NeuronCore per SEngine pair (devices 0,2,4,6).
# Measured WORSE (46.5 us vs 37.3): per-core DMA is capped ~215 GB/s even
# with the paired core idle, so fewer cores just means more bytes per core.
FOUR_CORES = False
PB = 2          # batch shards
PJ = 2 if FOUR_CORES else 4   # j shards
BL = B // PB    # 16 batches per core
JL = N // PJ    # output nodes per core (600 / 300)
NL = LAG - 1    # 12
CA = NL * BL    # 192 cols: T_A block, col = la*BL + b
CU = LAG * BL   # 208 cols: U block,  col = CA + t*BL + b
C = CA + CU     # 400 matmul moving cols
KT = 120        # contraction tile
NK = N // KT    # 10
JS = 100        # j subtile (psum partition dim)
NJ = JL // JS   # j subtiles per core (6 / 3)
NCH = 2         # input DMA chunks per tensor (5 k-tiles each)
USE_BF16 = False  # matmul operand dtype: bf16 halves DMA bytes, ~2.5e-3 rel err
DEVICE_IDS = [0, 2, 4, 6] if FOUR_CORES else list(range(8))

_cached = None


def _gamma(x):
    return math.gamma(x)


def _build_nc():
    import concourse.bacc as bacc
    import concourse.mybir as mybir
    from concourse.tile import TileContext

    f32 = mybir.dt.float32
    f32r = mybir.dt.bfloat16 if USE_BF16 else mybir.dt.float32r
    nc = bacc.Bacc(None, target_bir_lowering=False)

    # partition-major repacked inputs (see kernel() for layouts)
    xt = nc.dram_tensor("xt", [KT, NK * C], f32r, kind="ExternalInput")
    wc = nc.dram_tensor("wc", [KT, NK * JL], f32r, kind="ExternalInput")
    coef = nc.dram_tensor("coef", [JS, 3 * NJ * NL], f32, kind="ExternalInput")
    out = nc.dram_tensor("out", [JL, CA], f32, kind="ExternalOutput")

    with TileContext(nc) as tc:
        with (
            tc.tile_pool(name="sb", bufs=1) as pool,
            tc.tile_pool(name="ps", bufs=1, space="PSUM") as pspool,
        ):
            x_tiles = [None] * NK   # per-k [KT, C] views
            w_tiles = [None] * NK   # 8-core mode: per-k [KT, JL] views
            wjt = [None] * NJ       # 4-core mode: per-jt [KT, NK*JS] views

            def load(eng, kind, k0, nk, gi):
                dram, width, tl = (wc, JL, w_tiles) if kind == "w" \
                    else (xt, C, x_tiles)
                g = pool.tile([KT, nk * width], f32r, tag=f"g{gi}",
                              name=f"g{gi}")
                eng.dma_start(
                    out=g[:], in_=dram[:, k0 * width:(k0 + nk) * width])
                for kk in range(nk):
                    tl[k0 + kk] = g[:, kk * width:(kk + 1) * width]

            if FOUR_CORES:
                # X first on both queues (needed by every jt), then W
                # jt-major so per-jt epilogues pipeline with the W stream.
                # sync: Xk0-4 | Wjt0 | Wjt2 | Wjt4; scalar: Xk5-9 | Wjt1..5
                load(nc.sync, "x", 0, 5, 0)
                load(nc.scalar, "x", 5, 5, 1)
                KJ = NK * JS
                for jt in range(NJ):
                    g = pool.tile([KT, KJ], f32r, tag=f"wj{jt}",
                                  name=f"wj{jt}")
                    eng = nc.sync if jt % 2 == 0 else nc.scalar
                    eng.dma_start(out=g[:], in_=wc[:, jt * KJ:(jt + 1) * KJ])
                    wjt[jt] = g

                def w_slice(jt, k):
                    return wjt[jt][:, k * JS:(k + 1) * JS]
            else:
                # k-major on both queues with a per-jt staggered tail: W's
                # k8+k9 blocks are host-repacked per jt as contiguous 96 KB
                # pieces landing last, so jt0/jt1/jt2 finish ~0.5us apart
                # and each tanh+combine chain starts as soon as possible.
                # sync:   Wk0-2 | Wk3-5 | Xk6-7 | Xk8 | Xk9 | Wtail-jt0
                # scalar: Xk0-2 | Xk3-5 | Wk6-7 | Wtail-jt1 | Wtail-jt2
                load(nc.sync, "w", 0, 3, 0)
                load(nc.scalar, "x", 0, 3, 1)
                load(nc.sync, "w", 3, 3, 2)
                load(nc.scalar, "x", 3, 3, 3)
                load(nc.sync, "x", 6, 2, 4)
                load(nc.scalar, "w", 6, 2, 5)
                load(nc.sync, "x", 8, 1, 6)
                load(nc.scalar, "x", 9, 1, 7)
                wtail = []
                for jt in range(NJ):
                    g = pool.tile([KT, 2 * JS], f32r, tag=f"wt{jt}",
                                  name=f"wt{jt}")
                    # jt0+jt2 on sync, jt1 on scalar: the two queues drain
                    # the three tail pieces in parallel instead of one queue
                    # serializing two of them.
                    eng = nc.scalar if jt == 1 else nc.sync
                    c0 = 8 * JL + jt * 2 * JS
                    eng.dma_start(out=g[:], in_=wc[:, c0:c0 + 2 * JS])
                    wtail.append(g)

                def w_slice(jt, k):
                    if k >= 8:
                        return wtail[jt][:, (k - 8) * JS:(k - 7) * JS]
                    return w_tiles[k][:, jt * JS:(jt + 1) * JS]
            coef_all = pool.tile([JS, 3 * NJ * NL], f32, tag="coef")
            nc.gpsimd.dma_start(out=coef_all[:], in_=coef[:, :])

            # Warm up the PE clock gate (HAM) with throwaway bf16 matmuls
            # while inputs stream: otherwise the ~22 matmuls left after the
            # last DMA chunk run at the cold 333 ns cadence instead of 180.
            bf16 = mybir.dt.bfloat16
            scratch = pool.tile([KT, C], bf16, tag="scr")
            nc.vector.memset(scratch[:], 0)
            psw = pspool.tile([JS, C], f32, tag="psw", name="psw")
            for i in range(36):
                nc.tensor.matmul(psw[:], scratch[:, 0:JS], scratch[:],
                                 start=True, stop=True)

            ps = [pspool.tile([JS, C], f32, tag=f"ps{jt}", name=f"ps{jt}")
                  for jt in range(NJ)]
            if FOUR_CORES:
                mm_order = [(jt, k) for jt in range(NJ) for k in range(NK)]
            else:
                # k0-7 k-outer, then per-jt (k8, k9) so jt completions
                # stagger with the per-jt W tail pieces.
                mm_order = [(jt, k) for k in range(8) for jt in range(NJ)]
                mm_order += [(jt, k) for jt in range(NJ) for k in (8, 9)]
            # Dummy-matmul gap fillers after these k-groups keep the PE busy
            # through mid-stream DMA stalls so HAM stays at 2.4 GHz (a >3.4us
            # idle window re-throttles it to 1.2 GHz).
            fill_after = {2: 8, 5: 8}
            for jt, k in mm_order:
                nc.tensor.matmul(
                    ps[jt][:], w_slice(jt, k), x_tiles[k],
                    start=(k == 0), stop=(k == NK - 1),
                )
                if jt == NJ - 1 and k in fill_after:
                    for _ in range(fill_after[k]):
                        nc.tensor.matmul(psw[:], scratch[:, 0:JS],
                                         scratch[:], start=True, stop=True)

            # Replicate the [JS, 12] coefficient vectors to [JS, 192] during
            # the DMA phase (DVE idle) so the combine ops run on flat APs.
            crep = pool.tile([JS, 3 * NJ * CA], f32, tag="crep")
            for i in range(3):
                for jt in range(NJ):
                    src = coef_all[:, i * NJ * NL + jt * NL:
                                   i * NJ * NL + (jt + 1) * NL]
                    dst = crep[:, (i * NJ + jt) * CA:(i * NJ + jt + 1) * CA]
                    # DVE is idle during the DMA phase; GpSimd would contend
                    # for the shared DVE/GpSimd SBUF port later.
                    nc.vector.tensor_copy(
                        dst.rearrange("p (l b) -> p l b", b=BL),
                        src.broadcast_to([JS, NL, BL]))

            # Per-jt epilogue, pipelined: tanh on ACT, flat combine on DVE
            # (jt0, jt2) / GpSimd (jt1), per-jt output DMA.
            t_all = pool.tile([JS, NJ * C], f32, tag="t")
            res = pool.tile([JS, NJ * CA], f32, tag="res")
            tmp = pool.tile([JS, NJ * CA], f32, tag="tmp")
            tmp2 = pool.tile([JS, NJ * CA], f32, tag="tmp2")
            out3 = out.rearrange("(j p) c -> p j c", p=JS)
            for jt in range(NJ):
                # W was negated on the host, so psum = -(X @ W) already.
                nc.scalar.activation(
                    out=t_all[:, jt * C:(jt + 1) * C], in_=ps[jt][:],
                    func=mybir.ActivationFunctionType.Tanh,
                )
                t0 = jt * C
                tA = t_all[:, t0:t0 + CA]
                tU1 = t_all[:, t0 + CA + BL:t0 + CA + CU]
                tU0 = t_all[:, t0 + CA:t0 + CA + CA]
                rs = res[:, jt * CA:(jt + 1) * CA]
                ts = tmp[:, jt * CA:(jt + 1) * CA]
                ts2 = tmp2[:, jt * CA:(jt + 1) * CA]
                cof = [crep[:, (i * NJ + jt) * CA:(i * NJ + jt + 1) * CA]
                       for i in range(3)]
                # All combines on DVE: GpSimd shares the DVE SBUF port pair,
                # so "parallel" gp combines just halve both engines' rates.
                ve = nc.vector
                # three independent muls (pipeline on the engine), then adds
                ve.tensor_mul(rs, cof[0], tA)
                ve.tensor_mul(ts, cof[1], tU1)
                ve.tensor_mul(ts2, cof[2], tU0)
                ve.tensor_add(rs, rs, ts)
                ve.tensor_add(rs, rs, ts2)
                oeng = nc.sync if jt != 1 else nc.scalar
                oeng.dma_start(out=out3[:, jt, :], in_=rs)

    return nc


def _get_nc():
    global _cached
    if _cached is None:
        _cached = _build_nc()
        _cached.finalize()   # Bacc: runs reg alloc + codegen passes
    return _cached


def _host_coefs(alpha, fract, lambd, l):
    # All [12,...] fp32; compute in float64, cast at the end.
    a = alpha[:, 0].astype(np.float64)          # [12]
    f = fract[:, 0].astype(np.float64)          # [12]
    lam = lambd[:, 0, :, 0].astype(np.float64)  # [12, 200]
    ll = l[:, 0, :, 0].astype(np.float64)       # [12, 200]

    belta = np.zeros(NL)
    for la in range(NL):
        g_a1 = _gamma(a[la] + 1.0)
        belta[la] = sum(
            g_a1 / (_gamma(kk + 1.0) * _gamma(a[la] - kk + 1.0)) for kk in range(4)
        )
    cN = np.array([_gamma(a[la] + 1.0) / (6.0 * _gamma(a[la] - 2.0))
                   for la in range(NL)])

    # tile lambda/l from 200 -> 1200 (index n % 200)
    lam_t = np.tile(lam, (1, 6))                # [12, 1200]
    ll_t = np.tile(ll, (1, 6))                  # [12, 1200]

    inv_hf = (1.0 / H) ** f                     # 3**fract
    P = 2.0 * lam_t / belta[:, None] * inv_hf[:, None]
    Q = lam_t * ll_t / belta[:, None] / H
    R = Q * cN[:, None]
    return P.astype(np.float32), Q.astype(np.float32), R.astype(np.float32)


def _run_on_devices(nc, in_maps, device_ids):
    """run_bass_via_pjrt with an explicit device list (one core per SEngine
    pair) plus optional NTFF profiling. Returns a BassKernelResults."""
    import glob
    import os
    import tempfile

    import jax
    from jax.sharding import Mesh, PartitionSpec
    from jax.experimental.shard_map import shard_map

    import concourse.mybir as mybir
    from concourse.bass2jax import _bass_exec_p, install_neuronx_cc_hook
    from concourse.bass_utils import BassKernelResults, _process_ntff_profile

    install_neuronx_cc_hook()
    n_cores = len(device_ids)
    part_name = (nc.partition_id_tensor.name
                 if nc.partition_id_tensor else None)

    in_names, out_names, out_avals, zero_outs = [], [], [], []
    for alloc in nc.m.functions[0].allocations:
        if not isinstance(alloc, mybir.MemoryLocationSet):
            continue
        name = alloc.memorylocations[0].name
        if alloc.kind == "ExternalInput":
            if name != part_name:
                in_names.append(name)
        elif alloc.kind == "ExternalOutput":
            shape = tuple(alloc.tensor_shape)
            dtype = mybir.dt.np(alloc.dtype)
            out_names.append(name)
            out_avals.append(jax.core.ShapedArray(shape, dtype))
            zero_outs.append(np.zeros(shape, dtype))
    n_params = len(in_names)
    n_outs = len(out_avals)
    all_names = in_names + out_names
    if part_name is not None:
        all_names = all_names + [part_name]
    donate = tuple(range(n_params, n_params + n_outs))

    def _body(*args):
        operands = list(args)
        if part_name is not None:
            from concourse.bass2jax import partition_id_tensor
            operands.append(partition_id_tensor())
        outs = _bass_exec_p.bind(
            *operands,
            out_avals=tuple(out_avals),
            in_names=tuple(all_names),
            out_names=tuple(out_names),
            lowering_input_output_aliases=(),
            sim_require_finite=True,
            sim_require_nnan=True,
            nc=nc,
        )
        return tuple(outs)

    devices = [jax.devices()[i] for i in device_ids]
    mesh = Mesh(np.asarray(devices), ("core",))
    specs = (PartitionSpec("core"),) * (n_params + n_outs)
    sharded = jax.jit(
        shard_map(_body, mesh=mesh, in_specs=specs,
                  out_specs=(PartitionSpec("core"),) * n_outs,
                  check_rep=False),
        donate_argnums=donate, keep_unused=True,
    )
    concat_in = [
        np.concatenate([np.asarray(in_maps[c][nm]) for c in range(n_cores)],
                       axis=0) for nm in in_names
    ]
    concat_zeros = [
        np.zeros((n_cores * z.shape[0], *z.shape[1:]), z.dtype)
        for z in zero_outs
    ]

    trace = os.environ.get("BASS_TRACE") == "1"
    hook = None
    if trace:
        try:
            from antenv.axon_hooks import get_axon_ntff_profile_hook
            hook = get_axon_ntff_profile_hook()
        except ImportError:
            hook = None

    if hook is not None:
        neff_dir = tempfile.mkdtemp()
        with hook(neff_dir, [device_ids[0]]):
            out_arrs = sharded(*concat_in, *concat_zeros)
    else:
        out_arrs = sharded(*concat_in, *concat_zeros)

    results = [
        {nm: np.asarray(out_arrs[i]).reshape(n_cores, *out_avals[i].shape)[c]
         for i, nm in enumerate(out_names)}
        for c in range(n_cores)
    ]

    perf = BassKernelResults(results=results, instructions_and_trace=None,
                             profile_json=None, exec_time_ns=None)
    if hook is not None and glob.glob(os.path.join(neff_dir, "*_body*.ntff")):
        import gauge.profiler
        from concourse._compat import FishPath
        profile = gauge.profiler.Profile(
            profile_path=FishPath(neff_dir), kernel_dev_mode=True,
            profile_on_exit=False, bass_kernel=nc.m,
            offline_processing=True, fname="*_body*",
            metadata={"artifacts_path": neff_dir},
        )
        p = _process_ntff_profile(
            profile, neff_dir, nc, device_ids, [device_ids[0]], False, {},
            trace_events=False)
        perf = p.as_bass_kernel_results(results)
    return perf


def kernel(A, WW, train_init, alpha, fract, lambd, l, A_y_list):
    from concourse.bass_utils import run_bass_kernel_spmd

    if USE_BF16:
        import ml_dtypes
        mm_dt = ml_dtypes.bfloat16
    else:
        mm_dt = np.float32

    A = np.asarray(A, dtype=np.float32)
    WW = np.asarray(WW, dtype=np.float32)
    train_init = np.asarray(train_init, dtype=np.float32)

    P, Q, R = _host_coefs(
        np.asarray(alpha, np.float32), np.asarray(fract, np.float32),
        np.asarray(lambd, np.float32), np.asarray(l, np.float32))

    Wneg = -WW[:, :, 0]                         # [1200, 1200]

    xts, wcs, coefs = {}, {}, {}
    for beta in range(PB):
        bsl = slice(beta * BL, (beta + 1) * BL)
        xa = A[:, bsl, :, 0].transpose(2, 0, 1).reshape(N, CA)      # col=la*BL+b
        xu = train_init[bsl, :, :, 1].transpose(1, 2, 0).reshape(N, CU)  # col=t*BL+b
        XT = np.concatenate([xa, xu], axis=1)                       # [1200, 400]
        # partition-major: [KT, NK*C], col = k*C + c
        xts[beta] = np.ascontiguousarray(
            XT.reshape(NK, KT, C).transpose(1, 0, 2).reshape(KT, NK * C),
            dtype=mm_dt)
    for g in range(PJ):
        gsl = slice(g * JL, (g + 1) * JL)
        if FOUR_CORES:
            # partition-major, jt-major: col = jt*NK*JS + k*JS + s
            wcs[g] = np.ascontiguousarray(
                Wneg[:, gsl].reshape(NK, KT, NJ, JS).transpose(1, 2, 0, 3)
                .reshape(KT, NK * JL), dtype=mm_dt)
        else:
            # partition-major, k-major for k0-7, then per-jt (k8,k9) tails:
            # cols [k*JL + j for k<8] ++ [8*JL + jt*2*JS + (k-8)*JS + s]
            W3 = Wneg[:, gsl].reshape(NK, KT, JL)
            head = W3[:8].transpose(1, 0, 2).reshape(KT, 8 * JL)
            tails = [W3[k][:, jt * JS:(jt + 1) * JS]
                     for jt in range(NJ) for k in (8, 9)]
            wcs[g] = np.ascontiguousarray(
                np.concatenate([head] + tails, axis=1), dtype=mm_dt)
        # coef [JS, 108]: col = kind*36 + jt*12 + la
        kinds = [M[:, gsl].reshape(NL, NJ, JS).transpose(2, 1, 0)
                 for M in (P, Q, R)]                                # [100, 3, 12]
        coefs[g] = np.ascontiguousarray(
            np.stack(kinds, axis=1).reshape(JS, 3 * NJ * NL), dtype=np.float32)

    in_maps = []
    for core in range(PB * PJ):
        beta, g = divmod(core, PJ)
        in_maps.append({"xt": xts[beta], "wc": wcs[g], "coef": coefs[g]})

    nc = _get_nc()
    if FOUR_CORES:
        res = _run_on_devices(nc, in_maps, DEVICE_IDS)
    else:
        res = run_bass_kernel_spmd(nc, in_maps, core_ids=list(range(PB * PJ)))
    kernel.last_results = res

    full = np.empty((B, NL, N), dtype=np.float32)
    for core in range(PB * PJ):
        beta, g = divmod(core, PJ)
        o = res.results[core]["out"]            # [300, 192], col = la*BL+b
        full[beta * BL:(beta + 1) * BL, :, g * JL:(g + 1) * JL] = (
            o.reshape(JL, NL, BL).transpose(2, 1, 0))
    return full.reshape(B, NL, N, 1)


# revision 45
# speedup vs baseline: 1.3313x; 1.0204x over previous
# Trainium2 Bass kernel for nn_FCM_series_1 (gnn_message_passing).
#
# Math (derived from the reference):
#   aggregate(X, WW)[l,b,j] = tanh(-sum_i X[l,b,i] * WW[i,j])
#   T_A  = aggregate(A, WW)                     (12 lags x B rows)
#   U[t] = aggregate(train_init[:,:,t,1], WW)   (13 unique rows per batch;
#          A_N_OLD[la] = U[la], A_0_NEW[la] = U[la+1])
#   out[b,la,j] = P[la,j]*T_A[la,b,j] + Q[la,j]*U[la+1,b,j] + R[la,j]*U[la,b,j]
# with host-computable coefficients
#   P[la,j] = 2 * lambd[la, j%200] / belta[la] * 3**fract[la]
#   Q[la,j] = 3 * lambd[la, j%200] * l[la, j%200] / belta[la]
#   R[la,j] = Q[la,j] * Gamma(a+1)/(6*Gamma(a-2))
#   belta[la] = sum_{k=0..3} Gamma(a+1)/(Gamma(k+1)*Gamma(a-k+1))
#
# Sharding over 8 cores: batch split x2 (16 each), output node dim j split x4
# (300 each). Per core one matmul chain: lhsT=W-chunk tiles, rhs=X^T tiles,
# PSUM-accumulated over 10 k-tiles of 120, in float32r (single-pass fp32 PE
# mode, 4x faster than fp32 LOW_HIGH). W is negated on the host so psum
# already holds -X@W; tanh on ScalarE; coefficient combine on VectorE with
# 0-stride broadcast APs; per-core [300,192] result re-assembled on the host.
#
# HBM layouts are host-repacked to partition-major so every DMA descriptor is
# one large contiguous run per partition; input DMAs are split between the two
# HWDGE queues (sync for W, scalar for X) to double aggregate DMA throughput.

import math

import numpy as np

LAG = 13
B = 32
N = 1200
H = 1.0 / 3.0

# FOUR_CORES: run on one NeuronCore per SEngine pair (devices 0,2,4,6).
# Measured WORSE (46.5 us vs 37.3): per-core DMA is capped ~215 GB/s even
# with the paired core idle, so fewer cores just means more bytes per core.
FOUR_CORES = False
PB = 2          # batch shards
PJ = 2 if FOUR_CORES else 4   # j shards
BL = B // PB    # 16 batches per core
JL = N // PJ    # output nodes per core (600 / 300)
NL = LAG - 1    # 12
CA = NL * BL    # 192 cols: T_A block, col = la*BL + b
CU = LAG * BL   # 208 cols: U block,  col = CA + t*BL + b
C = CA + CU     # 400 matmul moving cols
KT = 120        # contraction tile
NK = N // KT    # 10
JS = 100        # j subtile (psum partition dim)
NJ = JL // JS   # j subtiles per core (6 / 3)
NCH = 2         # input DMA chunks per tensor (5 k-tiles each)
USE_BF16 = False  # matmul operand dtype: bf16 halves DMA bytes, ~2.5e-3 rel err
DEVICE_IDS = [0, 2, 4, 6] if FOUR_CORES else list(range(8))

_cached = None


def _gamma(x):
    return math.gamma(x)


def _build_nc():
    import concourse.bacc as bacc
    import concourse.mybir as mybir
    from concourse.tile import TileContext

    f32 = mybir.dt.float32
    f32r = mybir.dt.bfloat16 if USE_BF16 else mybir.dt.float32r
    nc = bacc.Bacc(None, target_bir_lowering=False)

    # partition-major repacked inputs (see kernel() for layouts)
    xt = nc.dram_tensor("xt", [KT, NK * C], f32r, kind="ExternalInput")
    wc = nc.dram_tensor("wc", [KT, NK * JL], f32r, kind="ExternalInput")
    coef = nc.dram_tensor("coef", [JS, 3 * NJ * NL], f32, kind="ExternalInput")
    out = nc.dram_tensor("out", [JL, CA], f32, kind="ExternalOutput")

    with TileContext(nc) as tc:
        with (
            tc.tile_pool(name="sb", bufs=1) as pool,
            tc.tile_pool(name="ps", bufs=1, space="PSUM") as pspool,
        ):
            x_tiles = [None] * NK   # per-k [KT, C] views
            w_tiles = [None] * NK   # 8-core mode: per-k [KT, JL] views
            wjt = [None] * NJ       # 4-core mode: per-jt [KT, NK*JS] views

            def load(eng, kind, k0, nk, gi):
                dram, width, tl = (wc, JL, w_tiles) if kind == "w" \
                    else (xt, C, x_tiles)
                g = pool.tile([KT, nk * width], f32r, tag=f"g{gi}",
                              name=f"g{gi}")
                eng.dma_start(
                    out=g[:], in_=dram[:, k0 * width:(k0 + nk) * width])
                for kk in range(nk):
                    tl[k0 + kk] = g[:, kk * width:(kk + 1) * width]

            if FOUR_CORES:
                # X first on both queues (needed by every jt), then W
                # jt-major so per-jt epilogues pipeline with the W stream.
                # sync: Xk0-4 | Wjt0 | Wjt2 | Wjt4; scalar: Xk5-9 | Wjt1..5
                load(nc.sync, "x", 0, 5, 0)
                load(nc.scalar, "x", 5, 5, 1)
                KJ = NK * JS
                for jt in range(NJ):
                    g = pool.tile([KT, KJ], f32r, tag=f"wj{jt}",
                                  name=f"wj{jt}")
                    eng = nc.sync if jt % 2 == 0 else nc.scalar
                    eng.dma_start(out=g[:], in_=wc[:, jt * KJ:(jt + 1) * KJ])
                    wjt[jt] = g

                def w_slice(jt, k):
                    return wjt[jt][:, k * JS:(k + 1) * JS]
            else:
                # k-major on both queues with a per-jt staggered tail: W's
                # k8+k9 blocks are host-repacked per jt as contiguous 96 KB
                # pieces landing last, so jt0/jt1/jt2 finish ~0.5us apart
                # and each tanh+combine chain starts as soon as possible.
                # sync:   Wk0-2 | Wk3-5 | Xk6-7 | Xk8 | Xk9 | Wtail-jt0
                # scalar: Xk0-2 | Xk3-5 | Wk6-7 | Wtail-jt1 | Wtail-jt2
                load(nc.sync, "w", 0, 3, 0)
                load(nc.scalar, "x", 0, 3, 1)
                load(nc.sync, "w", 3, 3, 2)
                load(nc.scalar, "x", 3, 3, 3)
                load(nc.sync, "x", 6, 2, 4)
                load(nc.scalar, "w", 6, 2, 5)
                load(nc.sync, "x", 8, 1, 6)
                load(nc.scalar, "x", 9, 1, 7)
                # The per-jt W tails ride the otherwise-idle gpsimd SWDGE
                # queue EARLY (288 KB, ~1.3% of traffic): as final items on
                # the HWDGE queues they each paid the ~1.5-2us DMA completion
                # latency in series. Now only Xk8/Xk9 land last, one receipt
                # per queue.
                wtail = []
                for jt in range(NJ):
                    g = pool.tile([KT, 2 * JS], f32r, tag=f"wt{jt}",
                                  name=f"wt{jt}")
                    c0 = 8 * JL + jt * 2 * JS
                    nc.gpsimd.dma_start(out=g[:], in_=wc[:, c0:c0 + 2 * JS])
                    wtail.append(g)

                def w_slice(jt, k):
                    if k >= 8:
                        return wtail[jt][:, (k - 8) * JS:(k - 7) * JS]
                    return w_tiles[k][:, jt * JS:(jt + 1) * JS]
            coef_all = pool.tile([JS, 3 * NJ * NL], f32, tag="coef")
            nc.gpsimd.dma_start(out=coef_all[:], in_=coef[:, :])

            # Warm up the PE clock gate (HAM) with throwaway bf16 matmuls
            # while inputs stream: otherwise the ~22 matmuls left after the
            # last DMA chunk run at the cold 333 ns cadence instead of 180.
            bf16 = mybir.dt.bfloat16
            scratch = pool.tile([KT, C], bf16, tag="scr")
            nc.vector.memset(scratch[:], 0)
            psw = pspool.tile([JS, C], f32, tag="psw", name="psw")
            for i in range(36):
                nc.tensor.matmul(psw[:], scratch[:, 0:JS], scratch[:],
                                 start=True, stop=True)

            ps = [pspool.tile([JS, C], f32, tag=f"ps{jt}", name=f"ps{jt}")
                  for jt in range(NJ)]
            if FOUR_CORES:
                mm_order = [(jt, k) for jt in range(NJ) for k in range(NK)]
            else:
                # k0-7 k-outer, then per-jt (k8, k9) so jt completions
                # stagger with the per-jt W tail pieces.
                mm_order = [(jt, k) for k in range(8) for jt in range(NJ)]
                mm_order += [(jt, k) for jt in range(NJ) for k in (8, 9)]
            # Dummy-matmul gap fillers after these k-groups keep the PE busy
            # through mid-stream DMA stalls so HAM stays at 2.4 GHz (a >3.4us
            # idle window re-throttles it to 1.2 GHz).
            fill_after = {2: 8, 5: 8}
            for jt, k in mm_order:
                nc.tensor.matmul(
                    ps[jt][:], w_slice(jt, k), x_tiles[k],
                    start=(k == 0), stop=(k == NK - 1),
                )
                if jt == NJ - 1 and k in fill_after:
                    for _ in range(fill_after[k]):
                        nc.tensor.matmul(psw[:], scratch[:, 0:JS],
                                         scratch[:], start=True, stop=True)

            # Replicate the [JS, 12] coefficient vectors to [JS, 192] during
            # the DMA phase (DVE idle) so the combine ops run on flat APs.
            crep = pool.tile([JS, 3 * NJ * CA], f32, tag="crep")
            for i in range(3):
                for jt in range(NJ):
                    src = coef_all[:, i * NJ * NL + jt * NL:
                                   i * NJ * NL + (jt + 1) * NL]
                    dst = crep[:, (i * NJ + jt) * CA:(i * NJ + jt + 1) * CA]
                    # DVE is idle during the DMA phase; GpSimd would contend
                    # for the shared DVE/GpSimd SBUF port later.
                    nc.vector.tensor_copy(
                        dst.rearrange("p (l b) -> p l b", b=BL),
                        src.broadcast_to([JS, NL, BL]))

            # Per-jt epilogue, pipelined: tanh on ACT, flat combine on DVE
            # (jt0, jt2) / GpSimd (jt1), per-jt output DMA.
            t_all = pool.tile([JS, NJ * C], f32, tag="t")
            res = pool.tile([JS, NJ * CA], f32, tag="res")
            tmp = pool.tile([JS, NJ * CA], f32, tag="tmp")
            tmp2 = pool.tile([JS, NJ * CA], f32, tag="tmp2")
            out3 = out.rearrange("(j p) c -> p j c", p=JS)
            for jt in range(NJ):
                # W was negated on the host, so psum = -(X @ W) already.
                nc.scalar.activation(
                    out=t_all[:, jt * C:(jt + 1) * C], in_=ps[jt][:],
                    func=mybir.ActivationFunctionType.Tanh,
                )
                t0 = jt * C
                tA = t_all[:, t0:t0 + CA]
                tU1 = t_all[:, t0 + CA + BL:t0 + CA + CU]
                tU0 = t_all[:, t0 + CA:t0 + CA + CA]
                rs = res[:, jt * CA:(jt + 1) * CA]
                ts = tmp[:, jt * CA:(jt + 1) * CA]
                ts2 = tmp2[:, jt * CA:(jt + 1) * CA]
                cof = [crep[:, (i * NJ + jt) * CA:(i * NJ + jt + 1) * CA]
                       for i in range(3)]
                # All combines on DVE: GpSimd shares the DVE SBUF port pair,
                # so "parallel" gp combines just halve both engines' rates.
                ve = nc.vector
                # three independent muls (pipeline on the engine), then adds
                ve.tensor_mul(rs, cof[0], tA)
                ve.tensor_mul(ts, cof[1], tU1)
                ve.tensor_mul(ts2, cof[2], tU0)
                ve.tensor_add(rs, rs, ts)
                ve.tensor_add(rs, rs, ts2)
                oeng = nc.sync if jt != 1 else nc.scalar
                oeng.dma_start(out=out3[:, jt, :], in_=rs)

    return nc


def _get_nc():
    global _cached
    if _cached is None:
        _cached = _build_nc()
        _cached.finalize()   # Bacc: runs reg alloc + codegen passes
    return _cached


def _host_coefs(alpha, fract, lambd, l):
    # All [12,...] fp32; compute in float64, cast at the end.
    a = alpha[:, 0].astype(np.float64)          # [12]
    f = fract[:, 0].astype(np.float64)          # [12]
    lam = lambd[:, 0, :, 0].astype(np.float64)  # [12, 200]
    ll = l[:, 0, :, 0].astype(np.float64)       # [12, 200]

    belta = np.zeros(NL)
    for la in range(NL):
        g_a1 = _gamma(a[la] + 1.0)
        belta[la] = sum(
            g_a1 / (_gamma(kk + 1.0) * _gamma(a[la] - kk + 1.0)) for kk in range(4)
        )
    cN = np.array([_gamma(a[la] + 1.0) / (6.0 * _gamma(a[la] - 2.0))
                   for la in range(NL)])

    # tile lambda/l from 200 -> 1200 (index n % 200)
    lam_t = np.tile(lam, (1, 6))                # [12, 1200]
    ll_t = np.tile(ll, (1, 6))                  # [12, 1200]

    inv_hf = (1.0 / H) ** f                     # 3**fract
    P = 2.0 * lam_t / belta[:, None] * inv_hf[:, None]
    Q = lam_t * ll_t / belta[:, None] / H
    R = Q * cN[:, None]
    return P.astype(np.float32), Q.astype(np.float32), R.astype(np.float32)


def _run_on_devices(nc, in_maps, device_ids):
    """run_bass_via_pjrt with an explicit device list (one core per SEngine
    pair) plus optional NTFF profiling. Returns a BassKernelResults."""
    import glob
    import os
    import tempfile

    import jax
    from jax.sharding import Mesh, PartitionSpec
    from jax.experimental.shard_map import shard_map

    import concourse.mybir as mybir
    from concourse.bass2jax import _bass_exec_p, install_neuronx_cc_hook
    from concourse.bass_utils import BassKernelResults, _process_ntff_profile

    install_neuronx_cc_hook()
    n_cores = len(device_ids)
    part_name = (nc.partition_id_tensor.name
                 if nc.partition_id_tensor else None)

    in_names, out_names, out_avals, zero_outs = [], [], [], []
    for alloc in nc.m.functions[0].allocations:
        if not isinstance(alloc, mybir.MemoryLocationSet):
            continue
        name = alloc.memorylocations[0].name
        if alloc.kind == "ExternalInput":
            if name != part_name:
                in_names.append(name)
        elif alloc.kind == "ExternalOutput":
            shape = tuple(alloc.tensor_shape)
            dtype = mybir.dt.np(alloc.dtype)
            out_names.append(name)
            out_avals.append(jax.core.ShapedArray(shape, dtype))
            zero_outs.append(np.zeros(shape, dtype))
    n_params = len(in_names)
    n_outs = len(out_avals)
    all_names = in_names + out_names
    if part_name is not None:
        all_names = all_names + [part_name]
    donate = tuple(range(n_params, n_params + n_outs))

    def _body(*args):
        operands = list(args)
        if part_name is not None:
            from concourse.bass2jax import partition_id_tensor
            operands.append(partition_id_tensor())
        outs = _bass_exec_p.bind(
            *operands,
            out_avals=tuple(out_avals),
            in_names=tuple(all_names),
            out_names=tuple(out_names),
            lowering_input_output_aliases=(),
            sim_require_finite=True,
            sim_require_nnan=True,
            nc=nc,
        )
        return tuple(outs)

    devices = [jax.devices()[i] for i in device_ids]
    mesh = Mesh(np.asarray(devices), ("core",))
    specs = (PartitionSpec("core"),) * (n_params + n_outs)
    sharded = jax.jit(
        shard_map(_body, mesh=mesh, in_specs=specs,
                  out_specs=(PartitionSpec("core"),) * n_outs,
                  check_rep=False),
        donate_argnums=donate, keep_unused=True,
    )
    concat_in = [
        np.concatenate([np.asarray(in_maps[c][nm]) for c in range(n_cores)],
                       axis=0) for nm in in_names
    ]
    concat_zeros = [
        np.zeros((n_cores * z.shape[0], *z.shape[1:]), z.dtype)
        for z in zero_outs
    ]

    trace = os.environ.get("BASS_TRACE") == "1"
    hook = None
    if trace:
        try:
            from antenv.axon_hooks import get_axon_ntff_profile_hook
            hook = get_axon_ntff_profile_hook()
        except ImportError:
            hook = None

    if hook is not None:
        neff_dir = tempfile.mkdtemp()
        with hook(neff_dir, [device_ids[0]]):
            out_arrs = sharded(*concat_in, *concat_zeros)
    else:
        out_arrs = sharded(*concat_in, *concat_zeros)

    results = [
        {nm: np.asarray(out_arrs[i]).reshape(n_cores, *out_avals[i].shape)[c]
         for i, nm in enumerate(out_names)}
        for c in range(n_cores)
    ]

    perf = BassKernelResults(results=results, instructions_and_trace=None,
                             profile_json=None, exec_time_ns=None)
    if hook is not None and glob.glob(os.path.join(neff_dir, "*_body*.ntff")):
        import gauge.profiler
        from concourse._compat import FishPath
        profile = gauge.profiler.Profile(
            profile_path=FishPath(neff_dir), kernel_dev_mode=True,
            profile_on_exit=False, bass_kernel=nc.m,
            offline_processing=True, fname="*_body*",
            metadata={"artifacts_path": neff_dir},
        )
        p = _process_ntff_profile(
            profile, neff_dir, nc, device_ids, [device_ids[0]], False, {},
            trace_events=False)
        perf = p.as_bass_kernel_results(results)
    return perf


def kernel(A, WW, train_init, alpha, fract, lambd, l, A_y_list):
    from concourse.bass_utils import run_bass_kernel_spmd

    if USE_BF16:
        import ml_dtypes
        mm_dt = ml_dtypes.bfloat16
    else:
        mm_dt = np.float32

    A = np.asarray(A, dtype=np.float32)
    WW = np.asarray(WW, dtype=np.float32)
    train_init = np.asarray(train_init, dtype=np.float32)

    P, Q, R = _host_coefs(
        np.asarray(alpha, np.float32), np.asarray(fract, np.float32),
        np.asarray(lambd, np.float32), np.asarray(l, np.float32))

    Wneg = -WW[:, :, 0]                         # [1200, 1200]

    xts, wcs, coefs = {}, {}, {}
    for beta in range(PB):
        bsl = slice(beta * BL, (beta + 1) * BL)
        xa = A[:, bsl, :, 0].transpose(2, 0, 1).reshape(N, CA)      # col=la*BL+b
        xu = train_init[bsl, :, :, 1].transpose(1, 2, 0).reshape(N, CU)  # col=t*BL+b
        XT = np.concatenate([xa, xu], axis=1)                       # [1200, 400]
        # partition-major: [KT, NK*C], col = k*C + c
        xts[beta] = np.ascontiguousarray(
            XT.reshape(NK, KT, C).transpose(1, 0, 2).reshape(KT, NK * C),
            dtype=mm_dt)
    for g in range(PJ):
        gsl = slice(g * JL, (g + 1) * JL)
        if FOUR_CORES:
            # partition-major, jt-major: col = jt*NK*JS + k*JS + s
            wcs[g] = np.ascontiguousarray(
                Wneg[:, gsl].reshape(NK, KT, NJ, JS).transpose(1, 2, 0, 3)
                .reshape(KT, NK * JL), dtype=mm_dt)
        else:
            # partition-major, k-major for k0-7, then per-jt (k8,k9) tails:
            # cols [k*JL + j for k<8] ++ [8*JL + jt*2*JS + (k-8)*JS + s]
            W3 = Wneg[:, gsl].reshape(NK, KT, JL)
            head = W3[:8].transpose(1, 0, 2).reshape(KT, 8 * JL)
            tails = [W3[k][:, jt * JS:(jt + 1) * JS]
                     for jt in range(NJ) for k in (8, 9)]
            wcs[g] = np.ascontiguousarray(
                np.concatenate([head] + tails, axis=1), dtype=mm_dt)
        # coef [JS, 108]: col = kind*36 + jt*12 + la
        kinds = [M[:, gsl].reshape(NL, NJ, JS).transpose(2, 1, 0)
                 for M in (P, Q, R)]                                # [100, 3, 12]
        coefs[g] = np.ascontiguousarray(
            np.stack(kinds, axis=1).reshape(JS, 3 * NJ * NL), dtype=np.float32)

    in_maps = []
    for core in range(PB * PJ):
        beta, g = divmod(core, PJ)
        in_maps.append({"xt": xts[beta], "wc": wcs[g], "coef": coefs[g]})

    nc = _get_nc()
    if FOUR_CORES:
        res = _run_on_devices(nc, in_maps, DEVICE_IDS)
    else:
        res = run_bass_kernel_spmd(nc, in_maps, core_ids=list(range(PB * PJ)))
    kernel.last_results = res

    full = np.empty((B, NL, N), dtype=np.float32)
    for core in range(PB * PJ):
        beta, g = divmod(core, PJ)
        o = res.results[core]["out"]            # [300, 192], col = la*BL+b
        full[beta * BL:(beta + 1) * BL, :, g * JL:(g + 1) * JL] = (
            o.reshape(JL, NL, BL).transpose(2, 1, 0))
    return full.reshape(B, NL, N, 1)
